# revision 1
# baseline (speedup 1.0000x reference)
"""Causal self-attention for B=4, L=2048, D=768, H=6 on 8 TRN2 NeuronCores.

Sharding: 8 cores = 4 batches x 2 head-groups (3 heads / 384 hidden each).
Each core computes, for its (batch, head-group):
  QT/KT = (x @ W{q,k})^T + b            [128d x L per head, fp32r]
  V     = x @ Wv                        [L x 384, fp32r]
  per head, per 512-wide q-group:
    S^T  = K_blk @ Q^T                  (PE, fp32r, causal block-skip)
    A^T  = exp(S^T / sqrt(128))         (ACT, masked on diagonal blocks)
    O^T += V_blk^T @ A^T                (PE)  + row-sums via ones-matmul
    O^T *= 1/sums  (sums broadcast over partitions via rank-1 matmul)
  Y_part = O @ Wo_slice                 [L x 768 partial]
Host sums the two head-group partials per batch and adds the bias terms
(bv @ Wo + bo); bq/bk are applied on-device (zero-cost per-partition add).

All matmuls run in float32r (full PE rate, ~1e-4 rel err); softmax math in
fp32. exp needs no max-subtraction: scores/sqrt(128) stay in [-10, 10] for
normally-distributed inputs, well inside fp32 exp range.
"""

import math

import numpy as np

import concourse.bacc as bacc
import concourse.mybir as mybir
import concourse.tile as tile
from concourse.bass_utils import run_bass_kernel_spmd

F32 = mybir.dt.float32
F32R = mybir.dt.float32r
EXP = mybir.ActivationFunctionType.Exp

B = 4
L = 2048
D = 768
HEADS = 6
HD = 128
HPC = 3          # heads per core
HG = HPC * HD    # 384: per-core slice of the hidden dim
CB = D // 128    # 6 contraction chunks
SCALE = 1.0 / math.sqrt(HD)
N_CORES = 8


def build_nc(L_=L):
    """Build + compile the per-core Bass program (same program on all cores)."""
    LBn = L_ // 128   # 128-row L blocks
    NQG = L_ // 512   # 512-wide q groups

    nc = bacc.Bacc("TRN2", target_bir_lowering=False, debug=False)
    x_d = nc.dram_tensor("x", [L_, D], F32, kind="ExternalInput").ap()
    wq_d = nc.dram_tensor("wq", [D, HG], F32, kind="ExternalInput").ap()
    wk_d = nc.dram_tensor("wk", [D, HG], F32, kind="ExternalInput").ap()
    wv_d = nc.dram_tensor("wv", [D, HG], F32, kind="ExternalInput").ap()
    wo_d = nc.dram_tensor("wo", [HG, D], F32, kind="ExternalInput").ap()
    bq_d = nc.dram_tensor("bq", [HG], F32, kind="ExternalInput").ap()
    bk_d = nc.dram_tensor("bk", [HG], F32, kind="ExternalInput").ap()
    ident_d = nc.dram_tensor("ident", [128, 128], F32, kind="ExternalInput").ap()
    maskf_d = nc.dram_tensor("maskf", [128, 896], F32, kind="ExternalInput").ap()
    y_d = nc.dram_tensor("y", [L_, D], F32, kind="ExternalOutput").ap()

    with tile.TileContext(nc) as tc:
        with (
            tc.tile_pool(name="persist", bufs=1) as pp,
            tc.tile_pool(name="qkv_sb", bufs=1) as pqkv,
        ):
            # constants go on the SWDGE (gpsimd) queue so the HWDGE queue's
            # first descriptors are the x chunks the PE transposes wait on
            ident = pp.tile([128, 128], F32R)
            nc.gpsimd.dma_start(ident, ident_d.bitcast(F32R))
            maskf = pp.tile([128, 896], F32R)
            bq_sb = pp.tile([128, HPC], F32)
            bk_sb = pp.tile([128, HPC], F32)
            nc.gpsimd.dma_start(bq_sb, bq_d.rearrange("(h p) -> p h", p=128))
            nc.gpsimd.dma_start(bk_sb, bk_d.rearrange("(h p) -> p h", p=128))
            # dummy exp: pulls the ACT Exp-table load off the QKV->attention
            # transition and into the startup DMA shadow
            warm = pp.tile([1, 1], F32)
            nc.scalar.activation(warm, ident[:1, :1], EXP)

            q_t = pqkv.tile([128, HPC, L_], F32R)   # Q^T: [d, (head, L)]
            k_t = pqkv.tile([128, HPC, L_], F32R)   # K^T
            v_t = pqkv.tile([128, LBn, HG], F32R)   # V:  [k-in-block, (block, hd)]
            o_t = pqkv.tile([128, HPC, L_], F32R)   # O^T (normalized)

            # ---- phase 1: load x, transpose to x^T, QKV projections ----
            with (
                tc.tile_pool(name="w_sb", bufs=1) as pw,
                tc.tile_pool(name="x_nat", bufs=8) as px,
                tc.tile_pool(name="xT", bufs=2) as pxt,
                tc.tile_pool(name="ps_t", bufs=2, space="PSUM") as ps_t,
                tc.tile_pool(name="ps_qk", bufs=2, space="PSUM") as ps_qk,
                tc.tile_pool(name="ps_v", bufs=2, space="PSUM") as ps_v,
            ):
                # per-128-row x tiles: fine-grained DMA→transpose pipelining
                def load_xb(g, b):
                    # alternate between the two HWDGE queues (SP / Activation)
                    # to parallelize descriptor generation and transfers
                    xb = px.tile([128, D], F32R, tag="xn")
                    r0 = g * 512 + b * 128
                    eng = nc.scalar if (g == 0 and b % 2 == 1) else nc.sync
                    eng.dma_start(
                        xb,
                        x_d.bitcast(F32R)[r0 : r0 + 128, :].rearrange(
                            "(o p) c -> p o c", p=128
                        )[:, 0],
                    )
                    return xb

                xbs = [load_xb(0, b) for b in range(4)]
                wq_sb = pw.tile([128, CB, HG], F32R)
                wk_sb = pw.tile([128, CB, HG], F32R)
                wv_sb = pw.tile([128, CB, HG], F32R)
                for w_sb, w_d in ((wq_sb, wq_d), (wk_sb, wk_d), (wv_sb, wv_d)):
                    nc.sync.dma_start(
                        w_sb, w_d.bitcast(F32R).rearrange("(c p) d -> p c d", p=128)
                    )

                def emit_transposes(g, xn):
                    # b-major groups: each PSUM group + copy depends on a
                    # single x row-block DMA, so the PE transposes stream in
                    # lockstep with the arriving sub-DMAs
                    xt = pxt.tile([128, CB, 512], F32R, name="xt")
                    for b in range(4):
                        for c0, cw in ((0, 4), (4, 2)):
                            pt = ps_t.tile([128, cw, 128], F32R, name="pt")
                            for ci in range(cw):
                                c = c0 + ci
                                nc.tensor.transpose(
                                    pt[:, ci, :],
                                    xn[b][:, c * 128 : (c + 1) * 128],
                                    ident,
                                )
                            nc.vector.tensor_copy(
                                xt[:, c0 : c0 + cw, b * 128 : (b + 1) * 128], pt
                            )
                    return xt

                xt = emit_transposes(0, xbs)
                for g in range(NQG):
                    if g + 1 < NQG:  # prefetch next chunk
                        xbs = [load_xb(g + 1, b) for b in range(4)]
                    qsl = slice(g * 512, (g + 1) * 512)
                    for h in range(HPC):
                        hsl = slice(h * 128, (h + 1) * 128)
                        pq = ps_qk.tile([128, 512], F32, tag="pq")
                        for c in range(CB):
                            nc.tensor.matmul(
                                pq, wq_sb[:, c, hsl], xt[:, c, :],
                                start=(c == 0), stop=(c == CB - 1),
                            )
                        nc.scalar.activation(
                            q_t[:, h, qsl], pq,
                            mybir.ActivationFunctionType.Identity,
                            bias=bq_sb[:, h : h + 1],
                        )
                        pk = ps_qk.tile([128, 512], F32, tag="pk")
                        for c in range(CB):
                            nc.tensor.matmul(
                                pk, wk_sb[:, c, hsl], xt[:, c, :],
                                start=(c == 0), stop=(c == CB - 1),
                            )
                        nc.scalar.activation(
                            k_t[:, h, qsl], pk,
                            mybir.ActivationFunctionType.Identity,
                            bias=bk_sb[:, h : h + 1],
                        )
                    # transposes for the next chunk run on the PE here, so
                    # their PSUM->SBUF copies land while the V matmuls run
                    xt_next = emit_transposes(g + 1, xbs) if g + 1 < NQG else None
                    for b in range(4):
                        lb = g * 4 + b
                        pv = ps_v.tile([128, HG], F32)
                        for c in range(CB):
                            nc.tensor.matmul(
                                pv, xt[:, c, b * 128 : (b + 1) * 128], wv_sb[:, c, :],
                                start=(c == 0), stop=(c == CB - 1),
                            )
                        nc.vector.tensor_copy(v_t[:, lb, :], pv)
                    xt = xt_next

            # ---- phase 2: attention + output projection ----
            with (
                tc.tile_pool(name="attn_sb", bufs=1) as pa,
                tc.tile_pool(name="at_pool", bufs=8) as pat,
                tc.tile_pool(name="nrm_sb", bufs=3) as pn,
                tc.tile_pool(name="y_pool", bufs=3) as py_,
                tc.tile_pool(name="ps_s", bufs=2, space="PSUM") as ps_s,
                tc.tile_pool(name="ps_o", bufs=2, space="PSUM") as ps_o,
                tc.tile_pool(name="ps_n", bufs=1, space="PSUM") as ps_n,
                tc.tile_pool(name="ps_y", bufs=1, space="PSUM") as ps_y,
            ):
                # maskf[p, c] = 1.0 if c >= p + 384 else 0.0; diagonal-block
                # mask for block i (0..3) is maskf[:, 384-128i : 896-128i].
                # maskf[:, 768:896] is all-ones: also used as the stationary
                # of the broadcast row-sum matmuls. Loaded here, off the
                # startup critical path.
                nc.sync.dma_start(maskf, maskf_d.bitcast(F32R))
                wo_sb = pa.tile([128, HPC, D], F32R)
                nc.sync.dma_start(
                    wo_sb, wo_d.bitcast(F32R).rearrange("(h p) e -> p h e", p=128)
                )
                # Flat software-pipelined stream over all (g, h, j) batches.
                # Per batch: S-matmuls -> exp (ACT) -> mask (DVE, diag only)
                # -> PV + row-sum matmuls. The S-matmuls of batch m+1 are
                # emitted before the PV of batch m, so the PE queue always
                # has an exp-independent batch in front of it, and the
                # finalize / projection work (which trails DVE results) is
                # emitted a batch or two late to avoid head-of-line blocks.
                flat = []
                for g in range(NQG):
                    nb = 2 * (g + 1)
                    order = list(range(nb))
                    for h in range(HPC):
                        for pos, j in enumerate(order):
                            flat.append((g, h, j, pos == nb - 1, pos == 0))
                state = {}
                pending = []  # (delay, closure)

                def emit_S(m):
                    g, h, j, last, first = flat[m]
                    ps = ps_s.tile([128, 2, 512], F32, tag="ps")
                    for t in range(2):
                        kb = 2 * j + t
                        i = kb - 4 * g
                        # diag block i: q-cols < 128i are fully masked -- skip
                        # them, but keep N >= 256 (fp32r below 256 drops to
                        # 4 cyc/row, costing more than the dead columns)
                        c0 = min(128 * i, 256) if i > 0 else 0
                        nc.tensor.matmul(
                            ps[:, t, c0:],
                            k_t[:, h, kb * 128 : (kb + 1) * 128],
                            q_t[:, h, g * 512 + c0 : (g + 1) * 512],
                            start=True, stop=True,
                        )
                    state[m] = ps

                def emit_rest(m):
                    g, h, j, last, first = flat[m]
                    ps = state.pop(m)
                    if first:
                        state[("po", g, h)] = ps_o.tile([128, 512], F32, tag="po", name="po")
                        state[("sm", g, h)] = ps_n.tile([128, 512], F32, tag="nrm", name="psums")
                    po = state[("po", g, h)]
                    psums = state[("sm", g, h)]
                    at = pat.tile([128, 2, 512], F32R)
                    diag = j >= 2 * g
                    if diag:
                        # per-t exp over just the computed columns
                        for t in range(2):
                            c0 = min(128 * (2 * j + t - 4 * g), 256)
                            nc.scalar.activation(
                                at[:, t, c0:], ps[:, t, c0:], EXP, scale=SCALE
                            )
                    elif last:
                        # split: halves the exp latency gating this group's
                        # finalize chain
                        nc.scalar.activation(at[:, 0, :], ps[:, 0, :], EXP, scale=SCALE)
                        nc.scalar.activation(at[:, 1, :], ps[:, 1, :], EXP, scale=SCALE)
                    else:
                        nc.scalar.activation(at, ps, EXP, scale=SCALE)
                    for t in range(2):
                        kb = 2 * j + t
                        i = kb - 4 * g
                        if i == 3:
                            # zero the computed-but-dead [256,384) plus the
                            # [384,512) triangle in one slice
                            nc.vector.tensor_mul(
                                at[:, t, 256:512], at[:, t, 256:512],
                                maskf[:, 256:512],
                            )
                        elif i >= 0:
                            # triangle mask on the diagonal 128-block; the
                            # dead cols below it are never computed or read
                            nc.vector.tensor_mul(
                                at[:, t, 128 * i : 128 * i + 128],
                                at[:, t, 128 * i : 128 * i + 128],
                                maskf[:, 384:512],
                            )
                        st, sp = first and t == 0, last and t == 1
                        c0 = min(128 * i, 256) if i > 0 else 0
                        nc.tensor.matmul(
                            po[:, c0:],
                            v_t[:, kb, h * 128 : (h + 1) * 128],
                            at[:, t, c0:],
                            start=st, stop=sp,
                        )
                        nc.tensor.matmul(
                            psums[:, c0:],
                            maskf[:, 768:896],
                            at[:, t, c0:],
                            start=st, stop=sp,
                        )

                def emit_finalize(g, h):
                    def run():
                        po = state.pop(("po", g, h))
                        psums = state.pop(("sm", g, h))
                        recip = pn.tile([128, 512], F32, tag="recip")
                        nc.vector.reciprocal(recip, psums)
                        nc.vector.tensor_mul(
                            o_t[:, h, g * 512 : (g + 1) * 512], po, recip
                        )
                    return run

                def emit_proj(g):
                    def run():
                        # the last group's projection runs exposed after all
                        # attention work; borrow the then-idle ps_s slots to
                        # triple-buffer it
                        final = g == NQG - 1
                        for b in range(4):
                            lb = g * 4 + b
                            lsl = slice(lb * 128, (lb + 1) * 128)
                            ysb = py_.tile([128, D], F32, tag="ysb")
                            for eh in range(2):
                                pool = ps_s if final and (b + eh) % 2 else ps_y
                                pyp = pool.tile(
                                    [128, 384], F32,
                                    tag="ps" if pool is ps_s else "pyp",
                                    name="pyp",
                                )
                                for h in range(HPC):
                                    nc.tensor.matmul(
                                        pyp,
                                        o_t[:, h, lsl],
                                        wo_sb[:, h, eh * 384 : (eh + 1) * 384],
                                        start=(h == 0), stop=(h == HPC - 1),
                                    )
                                nc.vector.tensor_copy(
                                    ysb[:, eh * 384 : (eh + 1) * 384], pyp
                                )
                            nc.sync.dma_start(y_d[lb * 128 : (lb + 1) * 128, :], ysb)
                    return run

                emit_S(0)
                for m in range(len(flat)):
                    if m + 1 < len(flat):
                        emit_S(m + 1)
                    nxt = []
                    for d, fn in pending:
                        if d <= 0:
                            fn()
                        else:
                            nxt.append((d - 1, fn))
                    pending = nxt
                    emit_rest(m)
                    g, h, j, last, first = flat[m]
                    if last:
                        pending.append((1, emit_finalize(g, h)))
                        if h == HPC - 1:
                            pending.append((2, emit_proj(g)))
                for d, fn in sorted(pending, key=lambda p: p[0]):
                    fn()

    nc.compile()
    return nc


_NC_CACHE = {}


def _get_nc(L_=L):
    if L_ not in _NC_CACHE:
        _NC_CACHE[L_] = build_nc(L_)
    return _NC_CACHE[L_]


def run_sharded(inputs, L_=L, trace=False):
    """Shard inputs over 8 cores, run, return (list of per-core y, results obj)."""
    x = np.ascontiguousarray(inputs["x_input"], dtype=np.float32)
    ident = np.eye(128, dtype=np.float32)
    maskf = (np.arange(896)[None, :] >= np.arange(128)[:, None] + 384).astype(
        np.float32
    )
    in_maps = []
    for c in range(N_CORES):
        b, gslice = c // 2, slice((c % 2) * HG, (c % 2) * HG + HG)
        in_maps.append(
            {
                "x": x[b],
                "ident": ident,
                "maskf": maskf,
                "wq": np.ascontiguousarray(inputs["Wq"][:, gslice], np.float32),
                "wk": np.ascontiguousarray(inputs["Wk"][:, gslice], np.float32),
                "wv": np.ascontiguousarray(inputs["Wv"][:, gslice], np.float32),
                "wo": np.ascontiguousarray(inputs["Wo"][gslice, :], np.float32),
                "bq": np.ascontiguousarray(inputs["bq"][gslice], np.float32),
                "bk": np.ascontiguousarray(inputs["bk"][gslice], np.float32),
            }
        )
    nc = _get_nc(L_)
    try:
        res = run_bass_kernel_spmd(nc, in_maps, list(range(N_CORES)), trace=trace)
    except Exception:
        # transient device faults (NRT_EXEC_UNIT_UNRECOVERABLE etc.): one retry
        res = run_bass_kernel_spmd(nc, in_maps, list(range(N_CORES)), trace=trace)
    return res


def kernel(**inputs) -> np.ndarray:
    res = run_sharded(inputs)
    # host-side unshard: sum the two head-group partials per batch; add the
    # bias terms that commute out of the device computation exactly:
    # softmax rows sum to 1, so  A @ (xWv + bv) Wo + bo = A(xWv)Wo + bv@Wo + bo
    bias = (
        np.asarray(inputs["bv"], np.float32) @ np.asarray(inputs["Wo"], np.float32)
        + np.asarray(inputs["bo"], np.float32)
    )
    out = np.empty((B, L, D), dtype=np.float32)
    for b in range(B):
        out[b] = res.results[2 * b]["y"] + res.results[2 * b + 1]["y"] + bias
    return out



# revision 20
# speedup vs baseline: 1.1499x; 1.1499x over previous
"""Causal self-attention for B=4, L=2048, D=768, H=6 on 8 TRN2 NeuronCores.

Sharding: 8 cores = 4 batches x 2 head-groups (3 heads / 384 hidden each).
All device math bf16 (fp32 PSUM accumulation), single merged pipeline:
the QKV projection of q-group g+1 runs as PE filler inside the attention
stream of q-group g, so the ACT exp latency/throughput never exposes PE
idle. x^T is pre-transposed on the host. Softmax denominators via DVE
bf16 accumulation + GPSIMD partition_all_reduce (no PE pass, no PSUM
bank); diagonal-block triangle masks on GPSIMD; projection staged
PSUM->SBUF (DVE, bf16) then DMA'd.
Host sums the two head-group partials per batch and adds bv@Wo + bo.
"""

import math

import numpy as np
import ml_dtypes

import concourse.bacc as bacc
import concourse.bass_isa as bass_isa
import concourse.mybir as mybir
import concourse.tile as tile
from concourse.bass_utils import run_bass_kernel_spmd

F32 = mybir.dt.float32
BF16 = mybir.dt.bfloat16
EXP = mybir.ActivationFunctionType.Exp
IDENT = mybir.ActivationFunctionType.Identity

B = 4
L = 2048
D = 768
HEADS = 6
HD = 128
HPC = 3          # heads per core
HG = HPC * HD    # 384: per-core slice of the hidden dim
CB = D // 128    # 6 contraction chunks
SCALE = 1.0 / math.sqrt(HD)
N_CORES = 8


def build_nc(L_=L):
    NQG = L_ // 512   # 512-wide q groups

    nc = bacc.Bacc("TRN2", target_bir_lowering=False, debug=False)
    xT_d = nc.dram_tensor("xT", [D, L_], BF16, kind="ExternalInput").ap()
    wq_d = nc.dram_tensor("wq", [D, HG], BF16, kind="ExternalInput").ap()
    wk_d = nc.dram_tensor("wk", [D, HG], BF16, kind="ExternalInput").ap()
    wv_d = nc.dram_tensor("wv", [D, HG], BF16, kind="ExternalInput").ap()
    wo_d = nc.dram_tensor("wo", [HG, D], BF16, kind="ExternalInput").ap()
    bq_d = nc.dram_tensor("bq", [HG], F32, kind="ExternalInput").ap()
    bk_d = nc.dram_tensor("bk", [HG], F32, kind="ExternalInput").ap()
    tri_d = nc.dram_tensor("tri", [128, 128], BF16, kind="ExternalInput").ap()
    y_d = nc.dram_tensor("y", [L_, D], BF16, kind="ExternalOutput").ap()

    xT_r = xT_d.rearrange("(c p) l -> p c l", p=128)

    with tile.TileContext(nc) as tc:
        with (
            tc.tile_pool(name="persist", bufs=1) as pp,
            tc.tile_pool(name="qkv_sb", bufs=1) as pqkv,
            tc.tile_pool(name="xT", bufs=3) as pxt,
            tc.tile_pool(name="at_pool", bufs=8) as pat,
            tc.tile_pool(name="acc_pool", bufs=3) as pacc,
            tc.tile_pool(name="z_pool", bufs=2) as pz,
            tc.tile_pool(name="nrm_sb", bufs=2) as pn,
            tc.tile_pool(name="ysb_pool", bufs=3) as pysb,
            tc.tile_pool(name="ps_g", bufs=3, space="PSUM") as ps_g,
            tc.tile_pool(name="ps_s", bufs=2, space="PSUM") as ps_s,
            tc.tile_pool(name="ps_o", bufs=1, space="PSUM") as ps_o,
        ):
            # small constants on the SWDGE (gpsimd) queue
            bq_sb = pp.tile([128, HPC], F32)
            bk_sb = pp.tile([128, HPC], F32)
            nc.gpsimd.dma_start(bq_sb, bq_d.rearrange("(h p) -> p h", p=128))
            nc.gpsimd.dma_start(bk_sb, bk_d.rearrange("(h p) -> p h", p=128))
            # dummy exp: pulls the ACT Exp-table load into the startup shadow
            warm = pp.tile([1, 1], F32)
            nc.scalar.activation(warm, bq_sb[:1, :1], EXP)

            q_t = pqkv.tile([128, HPC, L_], BF16)
            k_t = pqkv.tile([128, HPC, L_], BF16)
            v_t = pqkv.tile([128, L_ // 128, HG], BF16)
            o_t = pqkv.tile([128, HPC, L_], BF16)

            wq_sb = pp.tile([128, CB, HG], BF16)
            wk_sb = pp.tile([128, CB, HG], BF16)
            wv_sb = pp.tile([128, CB, HG], BF16)
            wo_sb = pp.tile([128, HPC, D], BF16)
            tri = pp.tile([128, 128], BF16)

            # ---- startup DMAs: wq/xt0 in fine chunks, rest whole ----
            xts = []
            xt0 = pxt.tile([128, CB, 512], BF16, tag="xt", name="xt")
            for c in range(CB):
                nc.scalar.dma_start(wq_sb[:, c, :], wq_d.rearrange(
                    "(c p) d -> p c d", p=128)[:, c, :])
                nc.sync.dma_start(xt0[:, c, :], xT_r[:, c, 0:512])
            xts.append(xt0)

            def issue_xt(g):
                xt = pxt.tile([128, CB, 512], BF16, tag="xt", name="xt")
                nc.sync.dma_start(xt, xT_r[:, :, g * 512 : (g + 1) * 512])
                return xt

            nc.scalar.dma_start(wk_sb, wk_d.rearrange("(c p) d -> p c d", p=128))
            xts.append(issue_xt(1))
            nc.scalar.dma_start(wv_sb, wv_d.rearrange("(c p) d -> p c d", p=128))
            xts.append(issue_xt(2))
            nc.scalar.dma_start(wo_sb, wo_d.rearrange("(h p) e -> p h e", p=128))
            nc.gpsimd.dma_start(tri, tri_d)

            # ---- phase-1 unit emitters ----
            def emit_qk_unit(g, h, which):
                w_sb, t_sb, b_sb = (
                    (wq_sb, q_t, bq_sb) if which == "q" else (wk_sb, k_t, bk_sb)
                )
                hsl = slice(h * 128, (h + 1) * 128)
                pq = ps_g.tile([128, 512], F32, tag="gemm", name="pg")
                for c in range(CB):
                    nc.tensor.matmul(
                        pq, w_sb[:, c, hsl], xts[g][:, c, :],
                        start=(c == 0), stop=(c == CB - 1),
                    )
                nc.scalar.activation(
                    t_sb[:, h, g * 512 : (g + 1) * 512], pq, IDENT,
                    bias=b_sb[:, h : h + 1],
                )

            def emit_v_unit(g, b):
                lb = g * 4 + b
                pv = ps_g.tile([128, 512], F32, tag="gemm", name="pg")
                for c in range(CB):
                    nc.tensor.matmul(
                        pv[:, :HG], xts[g][:, c, b * 128 : (b + 1) * 128],
                        wv_sb[:, c, :],
                        start=(c == 0), stop=(c == CB - 1),
                    )
                nc.vector.tensor_copy(v_t[:, lb, :], pv[:, :HG])

            def p1_units(g):
                units = []
                for h in range(HPC):
                    units.append(lambda g=g, h=h: emit_qk_unit(g, h, "q"))
                    units.append(lambda g=g, h=h: emit_qk_unit(g, h, "k"))
                for b in range(4):
                    units.append(lambda g=g, b=b: emit_v_unit(g, b))
                return units

            # ---- group 0 QKV: chunk-major q so matmuls start as soon as
            # the first wq/xT chunks land ----
            pqs = [ps_g.tile([128, 512], F32, tag="gemm", name="pg")
                   for _ in range(HPC)]
            for c in range(CB):
                for h in range(HPC):
                    nc.tensor.matmul(
                        pqs[h], wq_sb[:, c, h * 128 : (h + 1) * 128],
                        xts[0][:, c, :],
                        start=(c == 0), stop=(c == CB - 1),
                        skip_group_check=True,
                    )
            for h in range(HPC):
                nc.scalar.activation(
                    q_t[:, h, 0:512], pqs[h], IDENT, bias=bq_sb[:, h : h + 1]
                )
            del pqs
            for h in range(HPC):
                emit_qk_unit(0, h, "k")
            for b in range(4):
                emit_v_unit(0, b)

            # ---- merged attention + QKV(g+1) + projection stream ----
            flat = []
            for g in range(NQG):
                nb = 2 * (g + 1)
                for h in range(HPC):
                    for pos in range(nb):
                        flat.append((g, h, pos, pos == nb - 1, pos == 0))
            state = {}
            pending = []  # (delay_in_batches, closure)
            fillers = {}  # batch index -> list of closures

            # distribute QKV(g+1) units across attention batches of group g
            mstart = {}
            mi = 0
            for g in range(NQG):
                mstart[g] = mi
                mi += 2 * (g + 1) * HPC
            for g in range(NQG - 1):
                units = p1_units(g + 1)
                nbat = 2 * (g + 1) * HPC
                for j, u in enumerate(units):
                    m = mstart[g] + min(nbat - 1, (j * nbat) // len(units))
                    fillers.setdefault(m, []).append(u)
            # xt3 dma issued early in group-1's window (slot frees after
            # group-0's V units read xt0)
            fillers.setdefault(mstart[1], []).insert(
                0, lambda: xts.append(issue_xt(3))
            )

            def c0_of(g, kb):
                i = kb - 4 * g
                return 128 * i if i > 0 else 0

            def emit_S(m):
                g, h, j, last, first = flat[m]
                ps = ps_s.tile([128, 2, 512], F32, tag="ps")
                for t in range(2):
                    kb = 2 * j + t
                    c0 = 0 if j == 2 * g else c0_of(g, kb)
                    nc.tensor.matmul(
                        ps[:, t, c0:],
                        k_t[:, h, kb * 128 : (kb + 1) * 128],
                        q_t[:, h, g * 512 + c0 : (g + 1) * 512],
                        start=True, stop=True,
                    )
                state[m] = ps

            def emit_rest(m):
                g, h, j, last, first = flat[m]
                ps = state.pop(m)
                if first:
                    state[("po", g, h)] = ps_o.tile(
                        [128, 512], F32, tag="po", name="po"
                    )
                    state[("acc", g, h)] = pacc.tile(
                        [128, 512], BF16, tag="acc", name="acc"
                    )
                po = state[("po", g, h)]
                acc = state[("acc", g, h)]
                at = pat.tile([128, 2, 512], BF16, tag="at")
                if j == 2 * g + 1:
                    for t in range(2):
                        c0 = c0_of(g, 2 * j + t)
                        nc.scalar.activation(
                            at[:, t, c0:], ps[:, t, c0:], EXP, scale=SCALE
                        )
                else:
                    nc.scalar.activation(at, ps, EXP, scale=SCALE)
                for t in range(2):
                    kb = 2 * j + t
                    i = kb - 4 * g
                    c0 = c0_of(g, kb)
                    if i >= 0:
                        nc.gpsimd.tensor_mul(
                            at[:, t, c0 : c0 + 128], at[:, t, c0 : c0 + 128], tri
                        )
                    if first and t == 0:
                        nc.vector.tensor_copy(acc, at[:, 0, :])
                    else:
                        nc.vector.tensor_add(acc[:, c0:], acc[:, c0:], at[:, t, c0:])
                    nc.tensor.matmul(
                        po[:, c0:],
                        v_t[:, kb, h * 128 : (h + 1) * 128],
                        at[:, t, c0:],
                        start=(first and t == 0), stop=(last and t == 1),
                    )

            def emit_finalize(g, h):
                def run():
                    po = state.pop(("po", g, h))
                    acc = state.pop(("acc", g, h))
                    z = pz.tile([128, 512], F32, tag="z")
                    nc.gpsimd.partition_all_reduce(
                        z, acc, 128, bass_isa.ReduceOp.add
                    )
                    recip = pn.tile([128, 512], F32, tag="recip")
                    nc.vector.reciprocal(recip, z)
                    nc.vector.tensor_mul(
                        o_t[:, h, g * 512 : (g + 1) * 512], po, recip
                    )
                return run

            def proj_unit(g, b):
                def run():
                    lb = g * 4 + b
                    lsl = slice(lb * 128, (lb + 1) * 128)
                    ysb = pysb.tile([128, 2, 384], BF16, tag="ysb")
                    for eh in range(2):
                        pyp = ps_g.tile([128, 512], F32, tag="gemm", name="pg")
                        for h2 in range(HPC):
                            nc.tensor.matmul(
                                pyp[:, :384],
                                o_t[:, h2, lsl],
                                wo_sb[:, h2, eh * 384 : (eh + 1) * 384],
                                start=(h2 == 0), stop=(h2 == HPC - 1),
                            )
                        nc.vector.tensor_copy(ysb[:, eh, :], pyp[:, :384])
                    nc.sync.dma_start(
                        y_d[lb * 128 : (lb + 1) * 128, :].rearrange(
                            "p (u e) -> p u e", u=2
                        ),
                        ysb,
                    )
                return run

            emit_S(0)
            for m in range(len(flat)):
                if m + 1 < len(flat):
                    emit_S(m + 1)
                nxt = []
                for d, fn in pending:
                    if d <= 0:
                        fn()
                    else:
                        nxt.append((d - 1, fn))
                pending = nxt
                for u in fillers.get(m, ()):
                    u()
                emit_rest(m)
                g, h, j, last, first = flat[m]
                if last:
                    pending.append((1, emit_finalize(g, h)))
                    if h == HPC - 1:
                        for b in range(4):
                            pending.append((2 + b // 2, proj_unit(g, b)))
            for d, fn in sorted(pending, key=lambda p: p[0]):
                fn()

    nc.compile()
    return nc


_NC_CACHE = {}


def _get_nc(L_=L):
    if L_ not in _NC_CACHE:
        _NC_CACHE[L_] = build_nc(L_)
    return _NC_CACHE[L_]


def run_sharded(inputs, L_=L, trace=False):
    bf16 = ml_dtypes.bfloat16
    x = np.asarray(inputs["x_input"], dtype=np.float32).astype(bf16)
    xT = np.ascontiguousarray(x.transpose(0, 2, 1))  # [B, D, L]
    tri = np.triu(np.ones((128, 128), dtype=np.float32)).astype(bf16)
    wq = np.asarray(inputs["Wq"], np.float32).astype(bf16)
    wk = np.asarray(inputs["Wk"], np.float32).astype(bf16)
    wv = np.asarray(inputs["Wv"], np.float32).astype(bf16)
    wo = np.asarray(inputs["Wo"], np.float32).astype(bf16)
    bq = np.asarray(inputs["bq"], np.float32)
    bk = np.asarray(inputs["bk"], np.float32)
    in_maps = []
    for c in range(N_CORES):
        b, gslice = c // 2, slice((c % 2) * HG, (c % 2) * HG + HG)
        in_maps.append(
            {
                "xT": xT[b],
                "tri": tri,
                "wq": np.ascontiguousarray(wq[:, gslice]),
                "wk": np.ascontiguousarray(wk[:, gslice]),
                "wv": np.ascontiguousarray(wv[:, gslice]),
                "wo": np.ascontiguousarray(wo[gslice, :]),
                "bq": np.ascontiguousarray(bq[gslice]),
                "bk": np.ascontiguousarray(bk[gslice]),
            }
        )
    nc = _get_nc(L_)
    try:
        res = run_bass_kernel_spmd(nc, in_maps, list(range(N_CORES)), trace=trace)
    except Exception:
        res = run_bass_kernel_spmd(nc, in_maps, list(range(N_CORES)), trace=trace)
    return res


def kernel(**inputs) -> np.ndarray:
    res = run_sharded(inputs)
    bias = (
        np.asarray(inputs["bv"], np.float32) @ np.asarray(inputs["Wo"], np.float32)
        + np.asarray(inputs["bo"], np.float32)
    )
    out = np.empty((B, L, D), dtype=np.float32)
    for b in range(B):
        out[b] = (
            np.asarray(res.results[2 * b]["y"], dtype=np.float32)
            + np.asarray(res.results[2 * b + 1]["y"], dtype=np.float32)
            + bias
        )
    return out


# revision 37
# speedup vs baseline: 1.2781x; 1.1115x over previous
"""Causal self-attention for B=4, L=2048, D=768, H=6 on 8 TRN2 NeuronCores.

Sharding: 8 cores = 4 batches x 2 head-groups (3 heads / 384 hidden each).
All device math bf16 (fp32 PSUM accumulation), single merged pipeline:
the QKV projection of q-group g+1 runs as PE filler inside the attention
stream of q-group g, so the ACT exp latency/throughput never exposes PE
idle. x^T is pre-transposed on the host. Softmax denominators via DVE
bf16 accumulation + GPSIMD partition_all_reduce (no PE pass, no PSUM
bank); diagonal-block triangle masks on GPSIMD; projection staged
PSUM->SBUF (DVE, bf16) then DMA'd.
Host sums the two head-group partials per batch and adds bv@Wo + bo.
"""

import math

import numpy as np
import ml_dtypes

import concourse.bacc as bacc
import concourse.bass_isa as bass_isa
import concourse.mybir as mybir
import concourse.tile as tile
from concourse.bass_utils import run_bass_kernel_spmd

F32 = mybir.dt.float32
BF16 = mybir.dt.bfloat16
EXP = mybir.ActivationFunctionType.Exp
IDENT = mybir.ActivationFunctionType.Identity

B = 4
L = 2048
D = 768
HEADS = 6
HD = 128
HPC = 3          # heads per core
HG = HPC * HD    # 384: per-core slice of the hidden dim
CB = D // 128    # 6 contraction chunks
SCALE = 1.0 / math.sqrt(HD)
N_CORES = 8


def build_nc(L_=L):
    NQG = L_ // 512   # 512-wide q groups

    nc = bacc.Bacc("TRN2", target_bir_lowering=False, debug=False)
    xT_d = nc.dram_tensor("xT", [D, L_], BF16, kind="ExternalInput").ap()
    wq_d = nc.dram_tensor("wq", [D, HG], BF16, kind="ExternalInput").ap()
    wk_d = nc.dram_tensor("wk", [D, HG], BF16, kind="ExternalInput").ap()
    wv_d = nc.dram_tensor("wv", [D, HG], BF16, kind="ExternalInput").ap()
    wo_d = nc.dram_tensor("wo", [HG, D], BF16, kind="ExternalInput").ap()
    bq_d = nc.dram_tensor("bq", [HG], F32, kind="ExternalInput").ap()
    bk_d = nc.dram_tensor("bk", [HG], F32, kind="ExternalInput").ap()
    tri_d = nc.dram_tensor("tri", [128, 128], BF16, kind="ExternalInput").ap()
    y_d = nc.dram_tensor("y", [L_, D], BF16, kind="ExternalOutput").ap()

    xT_r = xT_d.rearrange("(c p) l -> p c l", p=128)

    with tile.TileContext(nc) as tc:
        with (
            tc.tile_pool(name="persist", bufs=1) as pp,
            tc.tile_pool(name="qkv_sb", bufs=1) as pqkv,
            tc.tile_pool(name="xT", bufs=3) as pxt,
            tc.tile_pool(name="at_pool", bufs=8) as pat,
            tc.tile_pool(name="acc_pool", bufs=3) as pacc,
            tc.tile_pool(name="z_pool", bufs=2) as pz,
            tc.tile_pool(name="nrm_sb", bufs=2) as pn,
            tc.tile_pool(name="ysb_pool", bufs=3) as pysb,
            tc.tile_pool(name="ps_g", bufs=2, space="PSUM") as ps_g,
            tc.tile_pool(name="ps_s", bufs=2, space="PSUM") as ps_s,
            tc.tile_pool(name="ps_o", bufs=2, space="PSUM") as ps_o,
        ):
            # small constants on the SWDGE (gpsimd) queue
            bq_sb = pp.tile([128, HPC], F32)
            bk_sb = pp.tile([128, HPC], F32)
            nc.gpsimd.dma_start(bq_sb, bq_d.rearrange("(h p) -> p h", p=128))
            nc.gpsimd.dma_start(bk_sb, bk_d.rearrange("(h p) -> p h", p=128))
            # dummy exp: pulls the ACT Exp-table load into the startup shadow
            warm = pp.tile([1, 1], F32)
            nc.scalar.activation(warm, bq_sb[:1, :1], EXP)

            q_t = pqkv.tile([128, HPC, L_], BF16)
            k_t = pqkv.tile([128, HPC, L_], BF16)
            v_t = pqkv.tile([128, L_ // 128, HG], BF16)
            o_t = pqkv.tile([128, HPC, L_], BF16)

            wq_sb = pp.tile([128, CB, HG], BF16)
            wk_sb = pp.tile([128, CB, HG], BF16)
            wv_sb = pp.tile([128, CB, HG], BF16)
            wo_sb = pp.tile([128, HPC, D], BF16)
            tri = pp.tile([128, 128], BF16)

            # ---- startup DMAs: wq/xt0 in halves, rest whole; wo deferred ----
            xts = []
            xt0 = pxt.tile([128, CB, 512], BF16, tag="xt", name="xt")
            wq_r = wq_d.rearrange("(c p) d -> p c d", p=128)
            for half in range(2):
                cs = slice(3 * half, 3 * half + 3)
                nc.scalar.dma_start(wq_sb[:, cs, :], wq_r[:, cs, :])
                nc.sync.dma_start(xt0[:, cs, :], xT_r[:, cs, 0:512])
            xts.append(xt0)

            def issue_xt(g):
                xt = pxt.tile([128, CB, 512], BF16, tag="xt", name="xt")
                nc.sync.dma_start(xt, xT_r[:, :, g * 512 : (g + 1) * 512])
                return xt

            nc.scalar.dma_start(wk_sb, wk_d.rearrange("(c p) d -> p c d", p=128))
            xts.append(issue_xt(1))
            nc.scalar.dma_start(wv_sb, wv_d.rearrange("(c p) d -> p c d", p=128))
            xts.append(issue_xt(2))
            nc.gpsimd.dma_start(tri, tri_d)

            # ---- phase-1 unit emitters ----
            def emit_qk_unit(g, h, which):
                w_sb, t_sb, b_sb = (
                    (wq_sb, q_t, bq_sb) if which == "q" else (wk_sb, k_t, bk_sb)
                )
                hsl = slice(h * 128, (h + 1) * 128)
                pq = ps_g.tile([128, 512], F32, tag="gemm", name="pg")
                for c in range(CB):
                    nc.tensor.matmul(
                        pq, w_sb[:, c, hsl], xts[g][:, c, :],
                        start=(c == 0), stop=(c == CB - 1),
                    )
                nc.scalar.activation(
                    t_sb[:, h, g * 512 : (g + 1) * 512], pq, IDENT,
                    bias=b_sb[:, h : h + 1],
                )

            def emit_v_unit(g, b):
                lb = g * 4 + b
                pv = ps_g.tile([128, 512], F32, tag="gemm", name="pg")
                for c in range(CB):
                    nc.tensor.matmul(
                        pv[:, :HG], xts[g][:, c, b * 128 : (b + 1) * 128],
                        wv_sb[:, c, :],
                        start=(c == 0), stop=(c == CB - 1),
                    )
                nc.vector.tensor_copy(v_t[:, lb, :], pv[:, :HG])

            def p1_units(g):
                units = []
                for h in range(HPC):
                    units.append(lambda g=g, h=h: emit_qk_unit(g, h, "q"))
                    units.append(lambda g=g, h=h: emit_qk_unit(g, h, "k"))
                for b in range(4):
                    units.append(lambda g=g, b=b: emit_v_unit(g, b))
                return units

            # PE warmup: dummy matmuls on a memset tile keep the tensor
            # engine's p-state ramp alive while the startup DMAs trickle in
            # (any PE idle gap resets the ramp to the slow p-state)
            scrap = pp.tile([128, 512], BF16)
            nc.vector.memset(scrap, 0.0)
            # scrap PSUM target: first slot of the (startup-idle) S ring
            ps_w = ps_s.tile([128, 2, 512], F32, tag="ps", name="warm")

            def pe_fill(n):
                for _ in range(n):
                    nc.tensor.matmul(
                        ps_w[:, 0, :], scrap[:, :128], scrap,
                        start=True, stop=True, skip_group_check=True,
                    )

            pe_fill(5)

            # ---- group 0 QKV: chunk-major q (2 heads) so matmuls start as
            # soon as the first wq/xT chunks land ----
            pqs = [ps_g.tile([128, 512], F32, tag="gemm", name="pg")
                   for _ in range(2)]
            for c in range(CB):
                if c == 3:
                    # bridge the gap until the second wq/xT halves land
                    pe_fill(2)
                for h in range(2):
                    nc.tensor.matmul(
                        pqs[h], wq_sb[:, c, h * 128 : (h + 1) * 128],
                        xts[0][:, c, :],
                        start=(c == 0), stop=(c == CB - 1),
                        skip_group_check=True,
                    )
                pe_fill(1)
            for h in range(2):
                nc.scalar.activation(
                    q_t[:, h, 0:512], pqs[h], IDENT, bias=bq_sb[:, h : h + 1]
                )
            del pqs
            pe_fill(2)
            emit_qk_unit(0, 2, "q")
            for h in range(HPC):
                emit_qk_unit(0, h, "k")
            for b in range(4):
                emit_v_unit(0, b)

            # ---- merged attention + QKV(g+1) + projection stream ----
            flat = []
            for g in range(NQG):
                nb = 2 * (g + 1)
                for h in range(HPC):
                    for pos in range(nb):
                        flat.append((g, h, pos, pos == nb - 1, pos == 0))
            state = {}
            pending = []  # (delay_in_batches, closure)
            fillers = {}  # batch index -> list of closures

            # distribute QKV(g+1) units across attention batches of group g
            mstart = {}
            mi = 0
            for g in range(NQG):
                mstart[g] = mi
                mi += 2 * (g + 1) * HPC
            for g in range(NQG - 1):
                units = p1_units(g + 1)
                nbat = 2 * (g + 1) * HPC
                for j, u in enumerate(units):
                    m = mstart[g] + min(nbat - 1, (j * nbat) // len(units))
                    fillers.setdefault(m, []).append(u)
            # wo load once the startup HWDGE burst has drained; xt3 early in
            # group-1's window (slot frees after group-0's V units read xt0)
            fillers.setdefault(mstart[0], []).insert(
                0,
                lambda: nc.sync.dma_start(
                    wo_sb, wo_d.rearrange("(h p) e -> p h e", p=128)
                ),
            )
            fillers.setdefault(mstart[1], []).insert(
                0, lambda: xts.append(issue_xt(3))
            )

            def c0_of(g, kb):
                i = kb - 4 * g
                return 128 * i if i > 0 else 0

            def emit_S(m):
                g, h, j, last, first = flat[m]
                ps = ps_s.tile([128, 2, 512], F32, tag="ps")
                for t in range(2):
                    kb = 2 * j + t
                    c0 = 0 if j == 2 * g else c0_of(g, kb)
                    nc.tensor.matmul(
                        ps[:, t, c0:],
                        k_t[:, h, kb * 128 : (kb + 1) * 128],
                        q_t[:, h, g * 512 + c0 : (g + 1) * 512],
                        start=True, stop=True,
                    )
                state[m] = ps

            def emit_rest(m):
                g, h, j, last, first = flat[m]
                ps = state.pop(m)
                if first:
                    state[("po", g, h)] = ps_o.tile(
                        [128, 512], F32, tag="po", name="po"
                    )
                    state[("acc", g, h)] = pacc.tile(
                        [128, 512], BF16, tag="acc", name="acc"
                    )
                po = state[("po", g, h)]
                acc = state[("acc", g, h)]
                at = pat.tile([128, 2, 512], BF16, tag="at")
                if j == 2 * g + 1:
                    for t in range(2):
                        c0 = c0_of(g, 2 * j + t)
                        nc.scalar.activation(
                            at[:, t, c0:], ps[:, t, c0:], EXP, scale=SCALE
                        )
                else:
                    nc.scalar.activation(at, ps, EXP, scale=SCALE)
                for t in range(2):
                    kb = 2 * j + t
                    i = kb - 4 * g
                    c0 = c0_of(g, kb)
                    if i >= 0:
                        nc.vector.tensor_mul(
                            at[:, t, c0 : c0 + 128], at[:, t, c0 : c0 + 128], tri
                        )
                    if first and t == 0:
                        nc.vector.tensor_copy(acc, at[:, 0, :])
                    else:
                        nc.vector.tensor_add(acc[:, c0:], acc[:, c0:], at[:, t, c0:])
                    nc.tensor.matmul(
                        po[:, c0:],
                        v_t[:, kb, h * 128 : (h + 1) * 128],
                        at[:, t, c0:],
                        start=(first and t == 0), stop=(last and t == 1),
                    )

            def emit_finalize(g, h):
                def run():
                    po = state.pop(("po", g, h))
                    acc = state.pop(("acc", g, h))
                    z = pz.tile([128, 512], F32, tag="z")
                    nc.gpsimd.partition_all_reduce(
                        z, acc, 128, bass_isa.ReduceOp.add
                    )
                    recip = pn.tile([128, 512], F32, tag="recip")
                    nc.vector.reciprocal(recip, z)
                    nc.vector.tensor_mul(
                        o_t[:, h, g * 512 : (g + 1) * 512], po, recip
                    )
                return run

            ysb_live = {}

            def proj_half(g, b, eh):
                def run():
                    lb = g * 4 + b
                    lsl = slice(lb * 128, (lb + 1) * 128)
                    if (g, b) not in ysb_live:
                        ysb_live[(g, b)] = pysb.tile(
                            [128, 2, 384], BF16, tag="ysb", name="ysb"
                        )
                    ysb = ysb_live[(g, b)]
                    pyp = ps_g.tile([128, 512], F32, tag="gemm", name="pg")
                    for h2 in range(HPC):
                        nc.tensor.matmul(
                            pyp[:, :384],
                            o_t[:, h2, lsl],
                            wo_sb[:, h2, eh * 384 : (eh + 1) * 384],
                            start=(h2 == 0), stop=(h2 == HPC - 1),
                        )
                    # alternate DVE/ACT so back-to-back projection copies
                    # run in parallel
                    if eh == 0:
                        nc.vector.tensor_copy(ysb[:, eh, :], pyp[:, :384])
                    else:
                        nc.scalar.activation(
                            ysb[:, eh, :], pyp[:, :384],
                            mybir.ActivationFunctionType.Copy,
                        )
                        nc.sync.dma_start(
                            y_d[lb * 128 : (lb + 1) * 128, :].rearrange(
                                "p (u e) -> p u e", u=2
                            ),
                            ysb,
                        )
                        del ysb_live[(g, b)]
                return run

            emit_S(0)
            for m in range(len(flat)):
                if m + 1 < len(flat):
                    emit_S(m + 1)
                nxt = []
                for d, fn in pending:
                    if d <= 0:
                        fn()
                    else:
                        nxt.append((d - 1, fn))
                pending = nxt
                for u in fillers.get(m, ()):
                    u()
                emit_rest(m)
                g, h, j, last, first = flat[m]
                if last:
                    pending.append((1, emit_finalize(g, h)))
                    if h == HPC - 1 and g < NQG - 1:
                        # spread the projection half-units across the next
                        # group's batches: they are the PE filler that
                        # absorbs the per-batch ACT exp overhead deficit
                        nnext = 2 * (g + 2) * HPC
                        for i, (b, eh) in enumerate(
                            (b, eh) for b in range(4) for eh in range(2)
                        ):
                            pending.append(
                                (2 + (i * (nnext - 4)) // 8, proj_half(g, b, eh))
                            )
            for d, fn in sorted(pending, key=lambda p: p[0]):
                fn()

            # ---- tail: last group's projection, split by head so the
            # h0/h1 partial matmuls run during the final softmax chain
            # (borrowing the now-idle S-ring PSUM banks) ----
            gl = NQG - 1
            pre = [(b, eh) for b in range(3) for eh in range(2)]
            tgts = []
            for _ in range(2):
                tile_s = ps_s.tile([128, 2, 512], F32, tag="ps", name="pyA")
                tgts += [tile_s[:, 0, :384], tile_s[:, 1, :384]]
            for _ in range(2):
                tile_g = ps_g.tile([128, 512], F32, tag="gemm", name="pg")
                tgts.append(tile_g[:, :384])
            for (b, eh), tg in zip(pre, tgts):
                lsl = slice((4 * gl + b) * 128, (4 * gl + b + 1) * 128)
                for h2 in (0, 1):
                    nc.tensor.matmul(
                        tg, o_t[:, h2, lsl],
                        wo_sb[:, h2, eh * 384 : (eh + 1) * 384],
                        start=(h2 == 0), stop=False,
                    )
            ysb4 = pp.tile([128, 4, 2, 384], BF16)

            def tail_copy(b, eh, tg):
                if eh == 0:
                    nc.vector.tensor_copy(ysb4[:, b, eh, :], tg)
                else:
                    nc.scalar.activation(
                        ysb4[:, b, eh, :], tg,
                        mybir.ActivationFunctionType.Copy,
                    )

            def tail_store(bpair):
                r0 = (4 * gl + 2 * bpair) * 128
                nc.sync.dma_start(
                    y_d[r0 : r0 + 256, :].rearrange(
                        "(b p) (u e) -> p b u e", p=128, u=2
                    ),
                    ysb4[:, 2 * bpair : 2 * bpair + 2],
                )

            for (b, eh), tg in zip(pre, tgts):
                lsl = slice((4 * gl + b) * 128, (4 * gl + b + 1) * 128)
                nc.tensor.matmul(
                    tg, o_t[:, 2, lsl], wo_sb[:, 2, eh * 384 : (eh + 1) * 384],
                    start=False, stop=True,
                )
                tail_copy(b, eh, tg)
                if b == 1 and eh == 1:
                    tail_store(0)
            # last row-block: full 3-matmul halves on recycled gemm slots
            for eh in range(2):
                pyl = ps_g.tile([128, 512], F32, tag="gemm", name="pg")
                lsl = slice((4 * gl + 3) * 128, (4 * gl + 4) * 128)
                for h2 in range(HPC):
                    nc.tensor.matmul(
                        pyl[:, :384], o_t[:, h2, lsl],
                        wo_sb[:, h2, eh * 384 : (eh + 1) * 384],
                        start=(h2 == 0), stop=(h2 == HPC - 1),
                    )
                tail_copy(3, eh, pyl[:, :384])
            tail_store(1)

    nc.compile()
    return nc


_NC_CACHE = {}


def _get_nc(L_=L):
    if L_ not in _NC_CACHE:
        _NC_CACHE[L_] = build_nc(L_)
    return _NC_CACHE[L_]


def run_sharded(inputs, L_=L, trace=False):
    bf16 = ml_dtypes.bfloat16
    x = np.asarray(inputs["x_input"], dtype=np.float32).astype(bf16)
    xT = np.ascontiguousarray(x.transpose(0, 2, 1))  # [B, D, L]
    tri = np.triu(np.ones((128, 128), dtype=np.float32)).astype(bf16)
    wq = np.asarray(inputs["Wq"], np.float32).astype(bf16)
    wk = np.asarray(inputs["Wk"], np.float32).astype(bf16)
    wv = np.asarray(inputs["Wv"], np.float32).astype(bf16)
    wo = np.asarray(inputs["Wo"], np.float32).astype(bf16)
    bq = np.asarray(inputs["bq"], np.float32)
    bk = np.asarray(inputs["bk"], np.float32)
    in_maps = []
    for c in range(N_CORES):
        b, gslice = c // 2, slice((c % 2) * HG, (c % 2) * HG + HG)
        in_maps.append(
            {
                "xT": xT[b],
                "tri": tri,
                "wq": np.ascontiguousarray(wq[:, gslice]),
                "wk": np.ascontiguousarray(wk[:, gslice]),
                "wv": np.ascontiguousarray(wv[:, gslice]),
                "wo": np.ascontiguousarray(wo[gslice, :]),
                "bq": np.ascontiguousarray(bq[gslice]),
                "bk": np.ascontiguousarray(bk[gslice]),
            }
        )
    nc = _get_nc(L_)
    try:
        res = run_bass_kernel_spmd(nc, in_maps, list(range(N_CORES)), trace=trace)
    except Exception:
        res = run_bass_kernel_spmd(nc, in_maps, list(range(N_CORES)), trace=trace)
    return res


def kernel(**inputs) -> np.ndarray:
    res = run_sharded(inputs)
    bias = (
        np.asarray(inputs["bv"], np.float32) @ np.asarray(inputs["Wo"], np.float32)
        + np.asarray(inputs["bo"], np.float32)
    )
    out = np.empty((B, L, D), dtype=np.float32)
    for b in range(B):
        out[b] = (
            np.asarray(res.results[2 * b]["y"], dtype=np.float32)
            + np.asarray(res.results[2 * b + 1]["y"], dtype=np.float32)
            + bias
        )
    return out


# revision 40
# speedup vs baseline: 1.2905x; 1.0097x over previous
"""Causal self-attention for B=4, L=2048, D=768, H=6 on 8 TRN2 NeuronCores.

Sharding: 8 cores = 4 batches x 2 head-groups (3 heads / 384 hidden each).
All device math bf16 (fp32 PSUM accumulation), single merged pipeline:
the QKV projection of q-group g+1 runs as PE filler inside the attention
stream of q-group g, so the ACT exp latency/throughput never exposes PE
idle. x^T is pre-transposed on the host. Softmax denominators via DVE
bf16 accumulation (2x perf mode) + GPSIMD partition_all_reduce (no PE
pass, no PSUM bank); diagonal-block triangle masks on DVE; projection
staged PSUM->SBUF (DVE/ACT alternating, bf16) then DMA'd; the last
group's projection is split by head so its h0/h1 partials overlap the
final softmax-normalization chain. Dummy warm-up matmuls bridge the
startup DMA latency so the PE p-state ramp is warm when real work lands.
Host sums the two head-group partials per batch and adds bv@Wo + bo
(softmax rows sum to 1, so the bv term commutes out exactly).
"""

import math

import numpy as np
import ml_dtypes

import concourse.bacc as bacc
import concourse.bass_isa as bass_isa
import concourse.mybir as mybir
import concourse.tile as tile
from concourse.bass_utils import run_bass_kernel_spmd

F32 = mybir.dt.float32
BF16 = mybir.dt.bfloat16
EXP = mybir.ActivationFunctionType.Exp
IDENT = mybir.ActivationFunctionType.Identity

B = 4
L = 2048
D = 768
HEADS = 6
HD = 128
HPC = 3          # heads per core
HG = HPC * HD    # 384: per-core slice of the hidden dim
CB = D // 128    # 6 contraction chunks
SCALE = 1.0 / math.sqrt(HD)
N_CORES = 8


def build_nc(L_=L):
    NQG = L_ // 512   # 512-wide q groups

    nc = bacc.Bacc("TRN2", target_bir_lowering=False, debug=False)
    xT_d = nc.dram_tensor("xT", [D, L_], BF16, kind="ExternalInput").ap()
    wq_d = nc.dram_tensor("wq", [D, HG], BF16, kind="ExternalInput").ap()
    wk_d = nc.dram_tensor("wk", [D, HG], BF16, kind="ExternalInput").ap()
    wv_d = nc.dram_tensor("wv", [D, HG], BF16, kind="ExternalInput").ap()
    wo_d = nc.dram_tensor("wo", [HG, D], BF16, kind="ExternalInput").ap()
    bq_d = nc.dram_tensor("bq", [HG], F32, kind="ExternalInput").ap()
    bk_d = nc.dram_tensor("bk", [HG], F32, kind="ExternalInput").ap()
    tri_d = nc.dram_tensor("tri", [128, 128], BF16, kind="ExternalInput").ap()
    y_d = nc.dram_tensor("y", [L_, D], BF16, kind="ExternalOutput").ap()

    xT_r = xT_d.rearrange("(c p) l -> p c l", p=128)

    with tile.TileContext(nc) as tc:
        with (
            tc.tile_pool(name="persist", bufs=1) as pp,
            tc.tile_pool(name="qkv_sb", bufs=1) as pqkv,
            tc.tile_pool(name="xT", bufs=3) as pxt,
            tc.tile_pool(name="at_pool", bufs=8) as pat,
            tc.tile_pool(name="acc_pool", bufs=3) as pacc,
            tc.tile_pool(name="z_pool", bufs=2) as pz,
            tc.tile_pool(name="nrm_sb", bufs=2) as pn,
            tc.tile_pool(name="ysb_pool", bufs=3) as pysb,
            tc.tile_pool(name="ps_g", bufs=2, space="PSUM") as ps_g,
            tc.tile_pool(name="ps_s", bufs=2, space="PSUM") as ps_s,
            tc.tile_pool(name="ps_o", bufs=2, space="PSUM") as ps_o,
        ):
            # small constants on the SWDGE (gpsimd) queue
            bq_sb = pp.tile([128, HPC], F32)
            bk_sb = pp.tile([128, HPC], F32)
            nc.gpsimd.dma_start(bq_sb, bq_d.rearrange("(h p) -> p h", p=128))
            nc.gpsimd.dma_start(bk_sb, bk_d.rearrange("(h p) -> p h", p=128))
            # dummy exp: pulls the ACT Exp-table load into the startup shadow
            warm = pp.tile([1, 1], F32)
            nc.scalar.activation(warm, bq_sb[:1, :1], EXP)

            q_t = pqkv.tile([128, HPC, L_], BF16)
            k_t = pqkv.tile([128, HPC, L_], BF16)
            v_t = pqkv.tile([128, L_ // 128, HG], BF16)
            o_t = pqkv.tile([128, HPC, L_], BF16)

            wq_sb = pp.tile([128, CB, HG], BF16)
            wk_sb = pp.tile([128, CB, HG], BF16)
            wv_sb = pp.tile([128, CB, HG], BF16)
            wo_sb = pp.tile([128, HPC, D], BF16)
            tri = pp.tile([128, 128], BF16)

            # ---- startup DMAs: wq/xt0 in halves, rest whole; wo deferred ----
            xts = []
            xt0 = pxt.tile([128, CB, 512], BF16, tag="xt", name="xt")
            wq_r = wq_d.rearrange("(c p) d -> p c d", p=128)
            for half in range(2):
                cs = slice(3 * half, 3 * half + 3)
                nc.scalar.dma_start(wq_sb[:, cs, :], wq_r[:, cs, :])
                nc.sync.dma_start(xt0[:, cs, :], xT_r[:, cs, 0:512])
            xts.append(xt0)

            def issue_xt(g):
                xt = pxt.tile([128, CB, 512], BF16, tag="xt", name="xt")
                nc.sync.dma_start(xt, xT_r[:, :, g * 512 : (g + 1) * 512])
                return xt

            nc.scalar.dma_start(wk_sb, wk_d.rearrange("(c p) d -> p c d", p=128))
            xts.append(issue_xt(1))
            nc.scalar.dma_start(wv_sb, wv_d.rearrange("(c p) d -> p c d", p=128))
            xts.append(issue_xt(2))
            nc.gpsimd.dma_start(tri, tri_d)

            # ---- phase-1 unit emitters ----
            def emit_qk_unit(g, h, which):
                w_sb, t_sb, b_sb = (
                    (wq_sb, q_t, bq_sb) if which == "q" else (wk_sb, k_t, bk_sb)
                )
                hsl = slice(h * 128, (h + 1) * 128)
                pq = ps_g.tile([128, 512], F32, tag="gemm", name="pg")
                for c in range(CB):
                    nc.tensor.matmul(
                        pq, w_sb[:, c, hsl], xts[g][:, c, :],
                        start=(c == 0), stop=(c == CB - 1),
                    )
                nc.scalar.activation(
                    t_sb[:, h, g * 512 : (g + 1) * 512], pq, IDENT,
                    bias=b_sb[:, h : h + 1],
                )

            def emit_v_unit(g, b):
                lb = g * 4 + b
                pv = ps_g.tile([128, 512], F32, tag="gemm", name="pg")
                for c in range(CB):
                    nc.tensor.matmul(
                        pv[:, :HG], xts[g][:, c, b * 128 : (b + 1) * 128],
                        wv_sb[:, c, :],
                        start=(c == 0), stop=(c == CB - 1),
                    )
                nc.vector.tensor_copy(v_t[:, lb, :], pv[:, :HG])

            def p1_units(g):
                units = []
                for h in range(HPC):
                    units.append(lambda g=g, h=h: emit_qk_unit(g, h, "q"))
                    units.append(lambda g=g, h=h: emit_qk_unit(g, h, "k"))
                for b in range(4):
                    units.append(lambda g=g, b=b: emit_v_unit(g, b))
                return units

            # PE warmup: dummy matmuls on a memset tile keep the tensor
            # engine's p-state ramp alive while the startup DMAs trickle in
            # (any PE idle gap resets the ramp to the slow p-state)
            scrap = pp.tile([128, 512], BF16)
            nc.vector.memset(scrap, 0.0)
            # scrap PSUM target: first slot of the (startup-idle) S ring
            ps_w = ps_s.tile([128, 2, 512], F32, tag="ps", name="warm")

            def pe_fill(n):
                for _ in range(n):
                    nc.tensor.matmul(
                        ps_w[:, 0, :], scrap[:, :128], scrap,
                        start=True, stop=True, skip_group_check=True,
                    )

            pe_fill(5)

            # ---- group 0 QKV: chunk-major q (2 heads) so matmuls start as
            # soon as the first wq/xT chunks land ----
            pqs = [ps_g.tile([128, 512], F32, tag="gemm", name="pg")
                   for _ in range(2)]
            for c in range(CB):
                if c == 3:
                    # bridge the gap until the second wq/xT halves land
                    pe_fill(2)
                for h in range(2):
                    nc.tensor.matmul(
                        pqs[h], wq_sb[:, c, h * 128 : (h + 1) * 128],
                        xts[0][:, c, :],
                        start=(c == 0), stop=(c == CB - 1),
                        skip_group_check=True,
                    )
                pe_fill(1)
            for h in range(2):
                nc.scalar.activation(
                    q_t[:, h, 0:512], pqs[h], IDENT, bias=bq_sb[:, h : h + 1]
                )
            del pqs
            pe_fill(2)
            emit_qk_unit(0, 2, "q")
            for h in range(HPC):
                emit_qk_unit(0, h, "k")
            for b in range(4):
                emit_v_unit(0, b)

            # ---- merged attention + QKV(g+1) + projection stream ----
            flat = []
            for g in range(NQG):
                nb = 2 * (g + 1)
                for h in range(HPC):
                    for pos in range(nb):
                        flat.append((g, h, pos, pos == nb - 1, pos == 0))
            state = {}
            pending = []  # (delay_in_batches, closure)
            fillers = {}  # batch index -> list of closures

            # distribute QKV(g+1) units across attention batches of group g
            mstart = {}
            mi = 0
            for g in range(NQG):
                mstart[g] = mi
                mi += 2 * (g + 1) * HPC
            for g in range(NQG - 1):
                units = p1_units(g + 1)
                nbat = 2 * (g + 1) * HPC
                for j, u in enumerate(units):
                    m = mstart[g] + min(nbat - 1, (j * nbat) // len(units))
                    fillers.setdefault(m, []).append(u)
            # wo load once the startup HWDGE burst has drained; xt3 early in
            # group-1's window (slot frees after group-0's V units read xt0)
            fillers.setdefault(mstart[0], []).insert(
                0,
                lambda: nc.sync.dma_start(
                    wo_sb, wo_d.rearrange("(h p) e -> p h e", p=128)
                ),
            )
            fillers.setdefault(mstart[1], []).insert(
                0, lambda: xts.append(issue_xt(3))
            )

            def c0_of(g, kb):
                i = kb - 4 * g
                return 128 * i if i > 0 else 0

            def emit_S(m):
                g, h, j, last, first = flat[m]
                ps = ps_s.tile([128, 2, 512], F32, tag="ps")
                for t in range(2):
                    kb = 2 * j + t
                    c0 = 0 if j == 2 * g else c0_of(g, kb)
                    nc.tensor.matmul(
                        ps[:, t, c0:],
                        k_t[:, h, kb * 128 : (kb + 1) * 128],
                        q_t[:, h, g * 512 + c0 : (g + 1) * 512],
                        start=True, stop=True,
                    )
                state[m] = ps

            def emit_rest(m):
                g, h, j, last, first = flat[m]
                ps = state.pop(m)
                if first:
                    state[("po", g, h)] = ps_o.tile(
                        [128, 512], F32, tag="po", name="po"
                    )
                    state[("acc", g, h)] = pacc.tile(
                        [128, 512], BF16, tag="acc", name="acc"
                    )
                po = state[("po", g, h)]
                acc = state[("acc", g, h)]
                at = pat.tile([128, 2, 512], BF16, tag="at")
                if j == 2 * g + 1:
                    for t in range(2):
                        c0 = c0_of(g, 2 * j + t)
                        nc.scalar.activation(
                            at[:, t, c0:], ps[:, t, c0:], EXP, scale=SCALE
                        )
                else:
                    nc.scalar.activation(at, ps, EXP, scale=SCALE)
                for t in range(2):
                    kb = 2 * j + t
                    i = kb - 4 * g
                    c0 = c0_of(g, kb)
                    if i >= 0:
                        nc.vector.tensor_mul(
                            at[:, t, c0 : c0 + 128], at[:, t, c0 : c0 + 128], tri
                        )
                    if first and t == 0:
                        nc.vector.tensor_copy(acc, at[:, 0, :])
                    else:
                        nc.vector.tensor_add(acc[:, c0:], acc[:, c0:], at[:, t, c0:])
                    nc.tensor.matmul(
                        po[:, c0:],
                        v_t[:, kb, h * 128 : (h + 1) * 128],
                        at[:, t, c0:],
                        start=(first and t == 0), stop=(last and t == 1),
                    )

            def emit_finalize(g, h):
                def run():
                    po = state.pop(("po", g, h))
                    acc = state.pop(("acc", g, h))
                    z = pz.tile([128, 512], F32, tag="z")
                    nc.gpsimd.partition_all_reduce(
                        z, acc, 128, bass_isa.ReduceOp.add
                    )
                    recip = pn.tile([128, 512], F32, tag="recip")
                    nc.vector.reciprocal(recip, z)
                    nc.vector.tensor_mul(
                        o_t[:, h, g * 512 : (g + 1) * 512], po, recip
                    )
                return run

            ysb_live = {}

            def proj_half(g, b, eh):
                def run():
                    lb = g * 4 + b
                    lsl = slice(lb * 128, (lb + 1) * 128)
                    if (g, b) not in ysb_live:
                        ysb_live[(g, b)] = pysb.tile(
                            [128, 2, 384], BF16, tag="ysb", name="ysb"
                        )
                    ysb = ysb_live[(g, b)]
                    pyp = ps_g.tile([128, 512], F32, tag="gemm", name="pg")
                    for h2 in range(HPC):
                        nc.tensor.matmul(
                            pyp[:, :384],
                            o_t[:, h2, lsl],
                            wo_sb[:, h2, eh * 384 : (eh + 1) * 384],
                            start=(h2 == 0), stop=(h2 == HPC - 1),
                        )
                    # alternate DVE/ACT so back-to-back projection copies
                    # run in parallel
                    if eh == 0:
                        nc.vector.tensor_copy(ysb[:, eh, :], pyp[:, :384])
                    else:
                        nc.scalar.activation(
                            ysb[:, eh, :], pyp[:, :384],
                            mybir.ActivationFunctionType.Copy,
                        )
                        nc.sync.dma_start(
                            y_d[lb * 128 : (lb + 1) * 128, :].rearrange(
                                "p (u e) -> p u e", u=2
                            ),
                            ysb,
                        )
                        del ysb_live[(g, b)]
                return run

            emit_S(0)
            for m in range(len(flat)):
                if m + 1 < len(flat):
                    emit_S(m + 1)
                nxt = []
                for d, fn in pending:
                    if d <= 0:
                        fn()
                    else:
                        nxt.append((d - 1, fn))
                pending = nxt
                for u in fillers.get(m, ()):
                    u()
                emit_rest(m)
                g, h, j, last, first = flat[m]
                if last:
                    pending.append((1, emit_finalize(g, h)))
                    if h == HPC - 1 and g < NQG - 1:
                        # spread the projection half-units across the next
                        # group's batches: they are the PE filler that
                        # absorbs the per-batch ACT exp overhead deficit
                        nnext = 2 * (g + 2) * HPC
                        for i, (b, eh) in enumerate(
                            (b, eh) for b in range(4) for eh in range(2)
                        ):
                            pending.append(
                                (2 + (i * (nnext - 4)) // 8, proj_half(g, b, eh))
                            )
            for d, fn in sorted(pending, key=lambda p: p[0]):
                fn()

            # ---- tail: last group's projection, split by head so the
            # h0/h1 partial matmuls run during the final softmax chain
            # (borrowing the now-idle S-ring PSUM banks) ----
            gl = NQG - 1
            pre = [(b, eh) for b in range(3) for eh in range(2)]
            tgts = []
            for _ in range(2):
                tile_s = ps_s.tile([128, 2, 512], F32, tag="ps", name="pyA")
                tgts += [tile_s[:, 0, :384], tile_s[:, 1, :384]]
            for _ in range(2):
                tile_g = ps_g.tile([128, 512], F32, tag="gemm", name="pg")
                tgts.append(tile_g[:, :384])
            for (b, eh), tg in zip(pre, tgts):
                lsl = slice((4 * gl + b) * 128, (4 * gl + b + 1) * 128)
                for h2 in (0, 1):
                    nc.tensor.matmul(
                        tg, o_t[:, h2, lsl],
                        wo_sb[:, h2, eh * 384 : (eh + 1) * 384],
                        start=(h2 == 0), stop=False,
                    )
            ysb4 = pp.tile([128, 4, 2, 384], BF16)

            def tail_copy(b, eh, tg):
                if eh == 0:
                    nc.vector.tensor_copy(ysb4[:, b, eh, :], tg)
                else:
                    nc.scalar.activation(
                        ysb4[:, b, eh, :], tg,
                        mybir.ActivationFunctionType.Copy,
                    )

            def tail_store(bpair, eng):
                r0 = (4 * gl + 2 * bpair) * 128
                eng.dma_start(
                    y_d[r0 : r0 + 256, :].rearrange(
                        "(b p) (u e) -> p b u e", p=128, u=2
                    ),
                    ysb4[:, 2 * bpair : 2 * bpair + 2],
                )

            for (b, eh), tg in zip(pre, tgts):
                lsl = slice((4 * gl + b) * 128, (4 * gl + b + 1) * 128)
                nc.tensor.matmul(
                    tg, o_t[:, 2, lsl], wo_sb[:, 2, eh * 384 : (eh + 1) * 384],
                    start=False, stop=True,
                )
                tail_copy(b, eh, tg)
                if b == 1 and eh == 1:
                    tail_store(0, nc.sync)
            # last row-block: full 3-matmul halves on recycled gemm slots
            for eh in range(2):
                pyl = ps_g.tile([128, 512], F32, tag="gemm", name="pg")
                lsl = slice((4 * gl + 3) * 128, (4 * gl + 4) * 128)
                for h2 in range(HPC):
                    nc.tensor.matmul(
                        pyl[:, :384], o_t[:, h2, lsl],
                        wo_sb[:, h2, eh * 384 : (eh + 1) * 384],
                        start=(h2 == 0), stop=(h2 == HPC - 1),
                    )
                tail_copy(3, eh, pyl[:, :384])
            tail_store(1, nc.scalar)

    nc.compile()
    return nc


_NC_CACHE = {}


def _get_nc(L_=L):
    if L_ not in _NC_CACHE:
        _NC_CACHE[L_] = build_nc(L_)
    return _NC_CACHE[L_]


def run_sharded(inputs, L_=L, trace=False):
    bf16 = ml_dtypes.bfloat16
    x = np.asarray(inputs["x_input"], dtype=np.float32).astype(bf16)
    xT = np.ascontiguousarray(x.transpose(0, 2, 1))  # [B, D, L]
    tri = np.triu(np.ones((128, 128), dtype=np.float32)).astype(bf16)
    wq = np.asarray(inputs["Wq"], np.float32).astype(bf16)
    wk = np.asarray(inputs["Wk"], np.float32).astype(bf16)
    wv = np.asarray(inputs["Wv"], np.float32).astype(bf16)
    wo = np.asarray(inputs["Wo"], np.float32).astype(bf16)
    bq = np.asarray(inputs["bq"], np.float32)
    bk = np.asarray(inputs["bk"], np.float32)
    in_maps = []
    for c in range(N_CORES):
        b, gslice = c // 2, slice((c % 2) * HG, (c % 2) * HG + HG)
        in_maps.append(
            {
                "xT": xT[b],
                "tri": tri,
                "wq": np.ascontiguousarray(wq[:, gslice]),
                "wk": np.ascontiguousarray(wk[:, gslice]),
                "wv": np.ascontiguousarray(wv[:, gslice]),
                "wo": np.ascontiguousarray(wo[gslice, :]),
                "bq": np.ascontiguousarray(bq[gslice]),
                "bk": np.ascontiguousarray(bk[gslice]),
            }
        )
    nc = _get_nc(L_)
    try:
        res = run_bass_kernel_spmd(nc, in_maps, list(range(N_CORES)), trace=trace)
    except Exception:
        res = run_bass_kernel_spmd(nc, in_maps, list(range(N_CORES)), trace=trace)
    return res


def kernel(**inputs) -> np.ndarray:
    res = run_sharded(inputs)
    bias = (
        np.asarray(inputs["bv"], np.float32) @ np.asarray(inputs["Wo"], np.float32)
        + np.asarray(inputs["bo"], np.float32)
    )
    out = np.empty((B, L, D), dtype=np.float32)
    for b in range(B):
        out[b] = (
            np.asarray(res.results[2 * b]["y"], dtype=np.float32)
            + np.asarray(res.results[2 * b + 1]["y"], dtype=np.float32)
            + bias
        )
    return out


# revision 46
# speedup vs baseline: 1.3000x; 1.0073x over previous
"""Causal self-attention for B=4, L=2048, D=768, H=6 on 8 TRN2 NeuronCores.

Sharding: 8 cores = 4 batches x 2 head-groups (3 heads / 384 hidden each).
All device math bf16 (fp32 PSUM accumulation), single merged pipeline:
the QKV projection of q-group g+1 runs as PE filler inside the attention
stream of q-group g, so the ACT exp latency/throughput never exposes PE
idle. x^T is pre-transposed on the host. Softmax denominators via DVE
bf16 accumulation (2x perf mode) + GPSIMD partition_all_reduce (no PE
pass, no PSUM bank); diagonal-block triangle masks on DVE; projection
staged PSUM->SBUF (DVE/ACT alternating, bf16) then DMA'd; the last
group's projection is split by head so its h0/h1 partials overlap the
final softmax-normalization chain. Dummy warm-up matmuls bridge the
startup DMA latency so the PE p-state ramp is warm when real work lands.
Host sums the two head-group partials per batch and adds bv@Wo + bo
(softmax rows sum to 1, so the bv term commutes out exactly).
"""

import math

import numpy as np
import ml_dtypes

import concourse.bacc as bacc
import concourse.bass_isa as bass_isa
import concourse.mybir as mybir
import concourse.tile as tile
from concourse.bass_utils import run_bass_kernel_spmd

F32 = mybir.dt.float32
BF16 = mybir.dt.bfloat16
EXP = mybir.ActivationFunctionType.Exp
IDENT = mybir.ActivationFunctionType.Identity

B = 4
L = 2048
D = 768
HEADS = 6
HD = 128
HPC = 3          # heads per core
HG = HPC * HD    # 384: per-core slice of the hidden dim
CB = D // 128    # 6 contraction chunks
SCALE = 1.0 / math.sqrt(HD)
N_CORES = 8


def build_nc(L_=L):
    NQG = L_ // 512   # 512-wide q groups

    nc = bacc.Bacc("TRN2", target_bir_lowering=False, debug=False)
    xT_d = nc.dram_tensor("xT", [D, L_], BF16, kind="ExternalInput").ap()
    wq_d = nc.dram_tensor("wq", [D, HG], BF16, kind="ExternalInput").ap()
    wk_d = nc.dram_tensor("wk", [D, HG], BF16, kind="ExternalInput").ap()
    wv_d = nc.dram_tensor("wv", [D, HG], BF16, kind="ExternalInput").ap()
    wo_d = nc.dram_tensor("wo", [HG, D], BF16, kind="ExternalInput").ap()
    bq_d = nc.dram_tensor("bq", [HG], F32, kind="ExternalInput").ap()
    bk_d = nc.dram_tensor("bk", [HG], F32, kind="ExternalInput").ap()
    tri_d = nc.dram_tensor("tri", [128, 128], BF16, kind="ExternalInput").ap()
    y_d = nc.dram_tensor("y", [L_, D], BF16, kind="ExternalOutput").ap()

    xT_r = xT_d.rearrange("(c p) l -> p c l", p=128)

    with tile.TileContext(nc) as tc:
        with (
            tc.tile_pool(name="persist", bufs=1) as pp,
            tc.tile_pool(name="qkv_sb", bufs=1) as pqkv,
            tc.tile_pool(name="xT", bufs=3) as pxt,
            tc.tile_pool(name="at_pool", bufs=8) as pat,
            tc.tile_pool(name="acc_pool", bufs=3) as pacc,
            tc.tile_pool(name="z_pool", bufs=2) as pz,
            tc.tile_pool(name="nrm_sb", bufs=2) as pn,
            tc.tile_pool(name="ysb_pool", bufs=3) as pysb,
            tc.tile_pool(name="ps_g", bufs=2, space="PSUM") as ps_g,
            tc.tile_pool(name="ps_s", bufs=2, space="PSUM") as ps_s,
            tc.tile_pool(name="ps_o", bufs=2, space="PSUM") as ps_o,
        ):
            # small constants on the SWDGE (gpsimd) queue
            bq_sb = pp.tile([128, HPC], F32)
            bk_sb = pp.tile([128, HPC], F32)
            nc.gpsimd.dma_start(bq_sb, bq_d.rearrange("(h p) -> p h", p=128))
            nc.gpsimd.dma_start(bk_sb, bk_d.rearrange("(h p) -> p h", p=128))
            # dummy exp: pulls the ACT Exp-table load into the startup shadow
            warm = pp.tile([1, 1], F32)
            nc.scalar.activation(warm, bq_sb[:1, :1], EXP)

            # per-head tensors as separate tiles: keeps the scheduler's
            # dependency tracking precise across heads
            q_ts = [pqkv.tile([128, L_], BF16, name=f"q_t{h}") for h in range(HPC)]
            k_ts = [pqkv.tile([128, L_], BF16, name=f"k_t{h}") for h in range(HPC)]
            v_t = pqkv.tile([128, L_ // 128, HG], BF16)
            o_ts = [pqkv.tile([128, L_], BF16, name=f"o_t{h}") for h in range(HPC)]

            wq_sb = pp.tile([128, CB, HG], BF16)
            wk_sb = pp.tile([128, CB, HG], BF16)
            wv_sb = pp.tile([128, CB, HG], BF16)
            wo_sb = pp.tile([128, HPC, D], BF16)
            tri = pp.tile([128, 128], BF16)

            # ---- startup DMAs: wq/xt0 in halves, rest whole; wo deferred ----
            xts = []
            xt0 = pxt.tile([128, CB, 512], BF16, tag="xt", name="xt")
            wq_r = wq_d.rearrange("(c p) d -> p c d", p=128)
            for half in range(2):
                cs = slice(3 * half, 3 * half + 3)
                nc.scalar.dma_start(wq_sb[:, cs, :], wq_r[:, cs, :])
                nc.sync.dma_start(xt0[:, cs, :], xT_r[:, cs, 0:512])
            xts.append(xt0)

            def issue_xt(g):
                xt = pxt.tile([128, CB, 512], BF16, tag="xt", name="xt")
                nc.sync.dma_start(xt, xT_r[:, :, g * 512 : (g + 1) * 512])
                return xt

            nc.scalar.dma_start(wk_sb, wk_d.rearrange("(c p) d -> p c d", p=128))
            xts.append(issue_xt(1))
            nc.scalar.dma_start(wv_sb, wv_d.rearrange("(c p) d -> p c d", p=128))
            xts.append(issue_xt(2))
            nc.gpsimd.dma_start(tri, tri_d)

            # ---- phase-1 unit emitters ----
            def emit_qk_unit(g, h, which):
                w_sb, t_sb, b_sb = (
                    (wq_sb, q_ts[h], bq_sb) if which == "q" else (wk_sb, k_ts[h], bk_sb)
                )
                hsl = slice(h * 128, (h + 1) * 128)
                pq = ps_g.tile([128, 512], F32, tag="gemm", name="pg")
                for c in range(CB):
                    nc.tensor.matmul(
                        pq, w_sb[:, c, hsl], xts[g][:, c, :],
                        start=(c == 0), stop=(c == CB - 1),
                    )
                nc.scalar.activation(
                    t_sb[:, g * 512 : (g + 1) * 512], pq, IDENT,
                    bias=b_sb[:, h : h + 1],
                )

            def emit_v_unit(g, b):
                lb = g * 4 + b
                pv = ps_g.tile([128, 512], F32, tag="gemm", name="pg")
                for c in range(CB):
                    nc.tensor.matmul(
                        pv[:, :HG], xts[g][:, c, b * 128 : (b + 1) * 128],
                        wv_sb[:, c, :],
                        start=(c == 0), stop=(c == CB - 1),
                    )
                nc.vector.tensor_copy(v_t[:, lb, :], pv[:, :HG])

            def p1_units(g):
                units = []
                for h in range(HPC):
                    units.append(lambda g=g, h=h: emit_qk_unit(g, h, "q"))
                    units.append(lambda g=g, h=h: emit_qk_unit(g, h, "k"))
                for b in range(4):
                    units.append(lambda g=g, b=b: emit_v_unit(g, b))
                return units

            # PE warmup: dummy matmuls on a memset tile keep the tensor
            # engine's p-state ramp alive while the startup DMAs trickle in
            # (any PE idle gap resets the ramp to the slow p-state)
            scrap = pp.tile([128, 512], BF16)
            nc.vector.memset(scrap, 0.0)
            # scrap PSUM target: first slot of the (startup-idle) S ring
            ps_w = ps_s.tile([128, 2, 512], F32, tag="ps", name="warm")

            def pe_fill(n):
                for _ in range(n):
                    nc.tensor.matmul(
                        ps_w[:, 0, :], scrap[:, :128], scrap,
                        start=True, stop=True, skip_group_check=True,
                    )

            pe_fill(5)

            # ---- group 0 QKV: chunk-major q (2 heads) so matmuls start as
            # soon as the first wq/xT chunks land ----
            pqs = [ps_g.tile([128, 512], F32, tag="gemm", name="pg")
                   for _ in range(2)]
            for c in range(CB):
                if c == 3:
                    # bridge the gap until the second wq/xT halves land
                    pe_fill(2)
                for h in range(2):
                    nc.tensor.matmul(
                        pqs[h], wq_sb[:, c, h * 128 : (h + 1) * 128],
                        xts[0][:, c, :],
                        start=(c == 0), stop=(c == CB - 1),
                        skip_group_check=True,
                    )
                pe_fill(1)
            for h in range(2):
                nc.scalar.activation(
                    q_ts[h][:, 0:512], pqs[h], IDENT, bias=bq_sb[:, h : h + 1]
                )
            del pqs
            pe_fill(2)
            emit_qk_unit(0, 2, "q")
            for h in range(HPC):
                emit_qk_unit(0, h, "k")
            for b in range(4):
                emit_v_unit(0, b)

            # ---- merged attention + QKV(g+1) + projection stream ----
            flat = []
            for g in range(NQG):
                nb = 2 * (g + 1)
                for h in range(HPC):
                    for pos in range(nb):
                        flat.append((g, h, pos, pos == nb - 1, pos == 0))
            state = {}
            pending = []  # (delay_in_batches, closure)
            fillers = {}  # batch index -> list of closures

            # distribute QKV(g+1) units across attention batches of group g
            mstart = {}
            mi = 0
            for g in range(NQG):
                mstart[g] = mi
                mi += 2 * (g + 1) * HPC
            for g in range(NQG - 1):
                units = p1_units(g + 1)
                nbat = 2 * (g + 1) * HPC
                for j, u in enumerate(units):
                    m = mstart[g] + min(nbat - 1, (j * nbat) // len(units))
                    fillers.setdefault(m, []).append(u)
            # wo load once the startup HWDGE burst has drained; xt3 early in
            # group-1's window (slot frees after group-0's V units read xt0)
            fillers.setdefault(mstart[0], []).insert(
                0,
                lambda: nc.sync.dma_start(
                    wo_sb, wo_d.rearrange("(h p) e -> p h e", p=128)
                ),
            )
            fillers.setdefault(mstart[1], []).insert(
                0, lambda: xts.append(issue_xt(3))
            )

            def c0_of(g, kb):
                i = kb - 4 * g
                return 128 * i if i > 0 else 0

            def emit_S(m):
                g, h, j, last, first = flat[m]
                ps = ps_s.tile([128, 2, 512], F32, tag="ps")
                for t in range(2):
                    kb = 2 * j + t
                    c0 = 0 if j == 2 * g else c0_of(g, kb)
                    nc.tensor.matmul(
                        ps[:, t, c0:],
                        k_ts[h][:, kb * 128 : (kb + 1) * 128],
                        q_ts[h][:, g * 512 + c0 : (g + 1) * 512],
                        start=True, stop=True,
                    )
                state[m] = ps

            def emit_rest(m):
                g, h, j, last, first = flat[m]
                ps = state.pop(m)
                if first:
                    state[("po", g, h)] = ps_o.tile(
                        [128, 512], F32, tag="po", name="po"
                    )
                    state[("acc", g, h)] = pacc.tile(
                        [128, 512], BF16, tag="acc", name="acc"
                    )
                po = state[("po", g, h)]
                acc = state[("acc", g, h)]
                at = pat.tile([128, 2, 512], BF16, tag="at")
                if j == 2 * g + 1:
                    for t in range(2):
                        c0 = c0_of(g, 2 * j + t)
                        nc.scalar.activation(
                            at[:, t, c0:], ps[:, t, c0:], EXP, scale=SCALE
                        )
                else:
                    nc.scalar.activation(at, ps, EXP, scale=SCALE)
                for t in range(2):
                    kb = 2 * j + t
                    i = kb - 4 * g
                    c0 = c0_of(g, kb)
                    if i >= 0:
                        nc.vector.tensor_mul(
                            at[:, t, c0 : c0 + 128], at[:, t, c0 : c0 + 128], tri
                        )
                    if first and t == 0:
                        nc.vector.tensor_copy(acc, at[:, 0, :])
                    else:
                        nc.vector.tensor_add(acc[:, c0:], acc[:, c0:], at[:, t, c0:])
                    nc.tensor.matmul(
                        po[:, c0:],
                        v_t[:, kb, h * 128 : (h + 1) * 128],
                        at[:, t, c0:],
                        start=(first and t == 0), stop=(last and t == 1),
                    )

            def emit_finalize(g, h, split=False):
                def run():
                    po = state.pop(("po", g, h))
                    acc = state.pop(("acc", g, h))
                    z = pz.tile([128, 512], F32, tag="z")
                    recip = pn.tile([128, 512], F32, tag="recip")
                    # split=True pipelines the chain in column halves so the
                    # tail projection can start after the first half
                    halves = ((0, 256), (256, 512)) if split else ((0, 512),)
                    for lo, hi in halves:
                        nc.gpsimd.partition_all_reduce(
                            z[:, lo:hi], acc[:, lo:hi], 128, bass_isa.ReduceOp.add
                        )
                        nc.vector.reciprocal(recip[:, lo:hi], z[:, lo:hi])
                        nc.vector.tensor_mul(
                            o_ts[h][:, g * 512 + lo : g * 512 + hi],
                            po[:, lo:hi], recip[:, lo:hi],
                        )
                return run

            ysb_live = {}

            def proj_half(g, b, eh):
                def run():
                    lb = g * 4 + b
                    lsl = slice(lb * 128, (lb + 1) * 128)
                    if (g, b) not in ysb_live:
                        ysb_live[(g, b)] = pysb.tile(
                            [128, 2, 384], BF16, tag="ysb", name="ysb"
                        )
                    ysb = ysb_live[(g, b)]
                    pyp = ps_g.tile([128, 512], F32, tag="gemm", name="pg")
                    for h2 in range(HPC):
                        nc.tensor.matmul(
                            pyp[:, :384],
                            o_ts[h2][:, lsl],
                            wo_sb[:, h2, eh * 384 : (eh + 1) * 384],
                            start=(h2 == 0), stop=(h2 == HPC - 1),
                        )
                    # alternate DVE/ACT so back-to-back projection copies
                    # run in parallel
                    if eh == 0:
                        nc.vector.tensor_copy(ysb[:, eh, :], pyp[:, :384])
                    else:
                        nc.scalar.activation(
                            ysb[:, eh, :], pyp[:, :384],
                            mybir.ActivationFunctionType.Copy,
                        )
                        nc.sync.dma_start(
                            y_d[lb * 128 : (lb + 1) * 128, :].rearrange(
                                "p (u e) -> p u e", u=2
                            ),
                            ysb,
                        )
                        del ysb_live[(g, b)]
                return run

            emit_S(0)
            for m in range(len(flat)):
                if m + 1 < len(flat):
                    emit_S(m + 1)
                nxt = []
                for d, fn in pending:
                    if d <= 0:
                        fn()
                    else:
                        nxt.append((d - 1, fn))
                pending = nxt
                for u in fillers.get(m, ()):
                    u()
                emit_rest(m)
                g, h, j, last, first = flat[m]
                if last:
                    pending.append((1, emit_finalize(
                        g, h, split=(g == NQG - 1 and h == HPC - 1))))
                    if h == HPC - 1 and g < NQG - 1:
                        # spread the projection half-units across the next
                        # group's batches: they are the PE filler that
                        # absorbs the per-batch ACT exp overhead deficit
                        nnext = 2 * (g + 2) * HPC
                        for i, (b, eh) in enumerate(
                            (b, eh) for b in range(4) for eh in range(2)
                        ):
                            pending.append(
                                (2 + (i * (nnext - 4)) // 8, proj_half(g, b, eh))
                            )
            # ---- tail: last group's projection, split by head so the
            # h0/h1 partial matmuls run during the final softmax chain
            # (borrowing the now-idle S-ring PSUM banks); emitted BEFORE
            # the flushed finalize so they sit earlier in the PE queue ----
            gl = NQG - 1
            pre = [(b, eh) for b in range(3) for eh in range(2)]
            tgts = []
            for _ in range(2):
                tile_s = ps_s.tile([128, 2, 512], F32, tag="ps", name="pyA")
                tgts += [tile_s[:, 0, :384], tile_s[:, 1, :384]]
            for _ in range(2):
                tile_g = ps_g.tile([128, 512], F32, tag="gemm", name="pg")
                tgts.append(tile_g[:, :384])
            for (b, eh), tg in zip(pre, tgts):
                lsl = slice((4 * gl + b) * 128, (4 * gl + b + 1) * 128)
                for h2 in (0, 1):
                    nc.tensor.matmul(
                        tg, o_ts[h2][:, lsl],
                        wo_sb[:, h2, eh * 384 : (eh + 1) * 384],
                        start=(h2 == 0), stop=False,
                    )

            for d, fn in sorted(pending, key=lambda p: p[0]):
                fn()
            ysb4 = pp.tile([128, 4, 2, 384], BF16)

            def tail_copy(b, eh, tg):
                if eh == 0:
                    nc.vector.tensor_copy(ysb4[:, b, eh, :], tg)
                else:
                    nc.scalar.activation(
                        ysb4[:, b, eh, :], tg,
                        mybir.ActivationFunctionType.Copy,
                    )

            def tail_store(bpair, eng):
                r0 = (4 * gl + 2 * bpair) * 128
                eng.dma_start(
                    y_d[r0 : r0 + 256, :].rearrange(
                        "(b p) (u e) -> p b u e", p=128, u=2
                    ),
                    ysb4[:, 2 * bpair : 2 * bpair + 2],
                )

            for (b, eh), tg in zip(pre, tgts):
                lsl = slice((4 * gl + b) * 128, (4 * gl + b + 1) * 128)
                nc.tensor.matmul(
                    tg, o_ts[2][:, lsl], wo_sb[:, 2, eh * 384 : (eh + 1) * 384],
                    start=False, stop=True,
                )
                tail_copy(b, eh, tg)
                if b == 1 and eh == 1:
                    tail_store(0, nc.sync)
                if b == 2 and eh == 1:
                    r2 = (4 * gl + 2) * 128
                    nc.sync.dma_start(
                        y_d[r2 : r2 + 128, :].rearrange("p (u e) -> p u e", u=2),
                        ysb4[:, 2],
                    )
            # last row-block: full 3-matmul halves on recycled gemm slots
            for eh in range(2):
                pyl = ps_g.tile([128, 512], F32, tag="gemm", name="pg")
                lsl = slice((4 * gl + 3) * 128, (4 * gl + 4) * 128)
                for h2 in range(HPC):
                    nc.tensor.matmul(
                        pyl[:, :384], o_ts[h2][:, lsl],
                        wo_sb[:, h2, eh * 384 : (eh + 1) * 384],
                        start=(h2 == 0), stop=(h2 == HPC - 1),
                    )
                tail_copy(3, eh, pyl[:, :384])
            r3 = (4 * gl + 3) * 128
            nc.scalar.dma_start(
                y_d[r3 : r3 + 128, :].rearrange("p (u e) -> p u e", u=2),
                ysb4[:, 3],
            )

    nc.compile()
    return nc


_NC_CACHE = {}


def _get_nc(L_=L):
    if L_ not in _NC_CACHE:
        _NC_CACHE[L_] = build_nc(L_)
    return _NC_CACHE[L_]


def run_sharded(inputs, L_=L, trace=False):
    bf16 = ml_dtypes.bfloat16
    x = np.asarray(inputs["x_input"], dtype=np.float32).astype(bf16)
    xT = np.ascontiguousarray(x.transpose(0, 2, 1))  # [B, D, L]
    tri = np.triu(np.ones((128, 128), dtype=np.float32)).astype(bf16)
    wq = np.asarray(inputs["Wq"], np.float32).astype(bf16)
    wk = np.asarray(inputs["Wk"], np.float32).astype(bf16)
    wv = np.asarray(inputs["Wv"], np.float32).astype(bf16)
    wo = np.asarray(inputs["Wo"], np.float32).astype(bf16)
    bq = np.asarray(inputs["bq"], np.float32)
    bk = np.asarray(inputs["bk"], np.float32)
    in_maps = []
    for c in range(N_CORES):
        b, gslice = c // 2, slice((c % 2) * HG, (c % 2) * HG + HG)
        in_maps.append(
            {
                "xT": xT[b],
                "tri": tri,
                "wq": np.ascontiguousarray(wq[:, gslice]),
                "wk": np.ascontiguousarray(wk[:, gslice]),
                "wv": np.ascontiguousarray(wv[:, gslice]),
                "wo": np.ascontiguousarray(wo[gslice, :]),
                "bq": np.ascontiguousarray(bq[gslice]),
                "bk": np.ascontiguousarray(bk[gslice]),
            }
        )
    nc = _get_nc(L_)
    try:
        res = run_bass_kernel_spmd(nc, in_maps, list(range(N_CORES)), trace=trace)
    except Exception:
        res = run_bass_kernel_spmd(nc, in_maps, list(range(N_CORES)), trace=trace)
    return res


def kernel(**inputs) -> np.ndarray:
    res = run_sharded(inputs)
    bias = (
        np.asarray(inputs["bv"], np.float32) @ np.asarray(inputs["Wo"], np.float32)
        + np.asarray(inputs["bo"], np.float32)
    )
    out = np.empty((B, L, D), dtype=np.float32)
    for b in range(B):
        out[b] = (
            np.asarray(res.results[2 * b]["y"], dtype=np.float32)
            + np.asarray(res.results[2 * b + 1]["y"], dtype=np.float32)
            + bias
        )
    return out


# revision 50
# speedup vs baseline: 1.3018x; 1.0014x over previous
"""Causal self-attention for B=4, L=2048, D=768, H=6 on 8 TRN2 NeuronCores.

Sharding: 8 cores = 4 batches x 2 head-groups (3 heads / 384 hidden each).
All device math bf16 (fp32 PSUM accumulation), single merged pipeline:
the QKV projection of q-group g+1 runs as PE filler inside the attention
stream of q-group g, so the ACT exp latency/throughput never exposes PE
idle. x^T is pre-transposed on the host. Softmax denominators via DVE
bf16 accumulation (2x perf mode) + GPSIMD partition_all_reduce (no PE
pass, no PSUM bank); diagonal-block triangle masks on DVE; projection
staged PSUM->SBUF (DVE/ACT alternating, bf16) then DMA'd; the last
group's projection is split by head so its h0/h1 partials overlap the
final softmax-normalization chain. Dummy warm-up matmuls bridge the
startup DMA latency so the PE p-state ramp is warm when real work lands.
Host sums the two head-group partials per batch and adds bv@Wo + bo
(softmax rows sum to 1, so the bv term commutes out exactly).
"""

import math

import numpy as np
import ml_dtypes

import concourse.bacc as bacc
import concourse.bass_isa as bass_isa
import concourse.mybir as mybir
import concourse.tile as tile
from concourse.bass_utils import run_bass_kernel_spmd

F32 = mybir.dt.float32
BF16 = mybir.dt.bfloat16
EXP = mybir.ActivationFunctionType.Exp
IDENT = mybir.ActivationFunctionType.Identity

B = 4
L = 2048
D = 768
HEADS = 6
HD = 128
HPC = 3          # heads per core
HG = HPC * HD    # 384: per-core slice of the hidden dim
CB = D // 128    # 6 contraction chunks
SCALE = 1.0 / math.sqrt(HD)
N_CORES = 8


def build_nc(L_=L):
    NQG = L_ // 512   # 512-wide q groups

    nc = bacc.Bacc("TRN2", target_bir_lowering=False, debug=False)
    xT_d = nc.dram_tensor("xT", [D, L_], BF16, kind="ExternalInput").ap()
    wq_d = nc.dram_tensor("wq", [D, HG], BF16, kind="ExternalInput").ap()
    wk_d = nc.dram_tensor("wk", [D, HG], BF16, kind="ExternalInput").ap()
    wv_d = nc.dram_tensor("wv", [D, HG], BF16, kind="ExternalInput").ap()
    wo_d = nc.dram_tensor("wo", [HG, D], BF16, kind="ExternalInput").ap()
    bq_d = nc.dram_tensor("bq", [HG], F32, kind="ExternalInput").ap()
    bk_d = nc.dram_tensor("bk", [HG], F32, kind="ExternalInput").ap()
    tri_d = nc.dram_tensor("tri", [128, 128], BF16, kind="ExternalInput").ap()
    y_d = nc.dram_tensor("y", [L_, D], BF16, kind="ExternalOutput").ap()

    xT_r = xT_d.rearrange("(c p) l -> p c l", p=128)

    with tile.TileContext(nc) as tc:
        with (
            tc.tile_pool(name="persist", bufs=1) as pp,
            tc.tile_pool(name="qkv_sb", bufs=1) as pqkv,
            tc.tile_pool(name="xT", bufs=3) as pxt,
            tc.tile_pool(name="at_pool", bufs=8) as pat,
            tc.tile_pool(name="acc_pool", bufs=3) as pacc,
            tc.tile_pool(name="z_pool", bufs=2) as pz,
            tc.tile_pool(name="nrm_sb", bufs=2) as pn,
            tc.tile_pool(name="ysb_pool", bufs=3) as pysb,
            tc.tile_pool(name="ps_g", bufs=2, space="PSUM") as ps_g,
            tc.tile_pool(name="ps_s", bufs=2, space="PSUM") as ps_s,
            tc.tile_pool(name="ps_o", bufs=2, space="PSUM") as ps_o,
        ):
            # small constants on the SWDGE (gpsimd) queue
            bq_sb = pp.tile([128, HPC], F32)
            bk_sb = pp.tile([128, HPC], F32)
            nc.gpsimd.dma_start(bq_sb, bq_d.rearrange("(h p) -> p h", p=128))
            nc.gpsimd.dma_start(bk_sb, bk_d.rearrange("(h p) -> p h", p=128))
            # dummy exp: pulls the ACT Exp-table load into the startup shadow
            warm = pp.tile([1, 1], F32)
            nc.scalar.activation(warm, bq_sb[:1, :1], EXP)

            # per-head tensors as separate tiles: keeps the scheduler's
            # dependency tracking precise across heads
            q_ts = [pqkv.tile([128, L_], BF16, name=f"q_t{h}") for h in range(HPC)]
            k_ts = [pqkv.tile([128, L_], BF16, name=f"k_t{h}") for h in range(HPC)]
            v_t = pqkv.tile([128, L_ // 128, HG], BF16)
            o_ts = [pqkv.tile([128, L_], BF16, name=f"o_t{h}") for h in range(HPC)]

            wq_sb = pp.tile([128, CB, HG], BF16)
            wk_sb = pp.tile([128, CB, HG], BF16)
            wv_sb = pp.tile([128, CB, HG], BF16)
            wo_sb = pp.tile([128, HPC, D], BF16)
            tri = pp.tile([128, 128], BF16)

            # ---- startup DMAs: wq/xt0 in halves, rest whole; wo deferred ----
            xts = []
            xt0 = pxt.tile([128, CB, 512], BF16, tag="xt", name="xt")
            wq_r = wq_d.rearrange("(c p) d -> p c d", p=128)
            for half in range(2):
                cs = slice(3 * half, 3 * half + 3)
                nc.scalar.dma_start(wq_sb[:, cs, :], wq_r[:, cs, :])
                nc.sync.dma_start(xt0[:, cs, :], xT_r[:, cs, 0:512])
            xts.append(xt0)

            def issue_xt(g):
                xt = pxt.tile([128, CB, 512], BF16, tag="xt", name="xt")
                nc.sync.dma_start(xt, xT_r[:, :, g * 512 : (g + 1) * 512])
                return xt

            nc.scalar.dma_start(wk_sb, wk_d.rearrange("(c p) d -> p c d", p=128))
            xts.append(issue_xt(1))
            nc.scalar.dma_start(wv_sb, wv_d.rearrange("(c p) d -> p c d", p=128))
            xts.append(issue_xt(2))
            nc.gpsimd.dma_start(tri, tri_d)

            # ---- phase-1 unit emitters ----
            def emit_qk_unit(g, h, which):
                w_sb, t_sb, b_sb = (
                    (wq_sb, q_ts[h], bq_sb) if which == "q" else (wk_sb, k_ts[h], bk_sb)
                )
                hsl = slice(h * 128, (h + 1) * 128)
                pq = ps_g.tile([128, 512], F32, tag="gemm", name="pg")
                for c in range(CB):
                    nc.tensor.matmul(
                        pq, w_sb[:, c, hsl], xts[g][:, c, :],
                        start=(c == 0), stop=(c == CB - 1),
                    )
                nc.scalar.activation(
                    t_sb[:, g * 512 : (g + 1) * 512], pq, IDENT,
                    bias=b_sb[:, h : h + 1],
                )

            def emit_v_unit(g, b):
                lb = g * 4 + b
                pv = ps_g.tile([128, 512], F32, tag="gemm", name="pg")
                for c in range(CB):
                    nc.tensor.matmul(
                        pv[:, :HG], xts[g][:, c, b * 128 : (b + 1) * 128],
                        wv_sb[:, c, :],
                        start=(c == 0), stop=(c == CB - 1),
                    )
                nc.vector.tensor_copy(v_t[:, lb, :], pv[:, :HG])

            def p1_units(g):
                units = []
                for h in range(HPC):
                    units.append(lambda g=g, h=h: emit_qk_unit(g, h, "q"))
                    units.append(lambda g=g, h=h: emit_qk_unit(g, h, "k"))
                for b in range(4):
                    units.append(lambda g=g, b=b: emit_v_unit(g, b))
                return units

            # PE warmup: dummy matmuls on a memset tile keep the tensor
            # engine's p-state ramp alive while the startup DMAs trickle in
            # (any PE idle gap resets the ramp to the slow p-state)
            scrap = pp.tile([128, 128], BF16)
            nc.vector.memset(scrap, 0.0)
            # scrap PSUM target: first slot of the (startup-idle) S ring
            ps_w = ps_s.tile([128, 2, 512], F32, tag="ps", name="warm")

            def pe_fill(n):
                # n counted in 512-col equivalents; emit 128-col dummies for
                # finer-grained bridging of the startup DMA pacing
                for _ in range(4 * n):
                    nc.tensor.matmul(
                        ps_w[:, 0, :128], scrap, scrap,
                        start=True, stop=True, skip_group_check=True,
                    )

            pe_fill(5)

            # ---- group 0 QKV: chunk-major q (2 heads) so matmuls start as
            # soon as the first wq/xT chunks land ----
            pqs = [ps_g.tile([128, 512], F32, tag="gemm", name="pg")
                   for _ in range(2)]
            for c in range(CB):
                if c == 3:
                    # bridge the gap until the second wq/xT halves land
                    pe_fill(2)
                for h in range(2):
                    nc.tensor.matmul(
                        pqs[h], wq_sb[:, c, h * 128 : (h + 1) * 128],
                        xts[0][:, c, :],
                        start=(c == 0), stop=(c == CB - 1),
                        skip_group_check=True,
                    )
                pe_fill(1)
            for h in range(2):
                nc.scalar.activation(
                    q_ts[h][:, 0:512], pqs[h], IDENT, bias=bq_sb[:, h : h + 1]
                )
            del pqs
            pe_fill(2)
            emit_qk_unit(0, 2, "q")
            for h in range(HPC):
                emit_qk_unit(0, h, "k")
            for b in range(4):
                emit_v_unit(0, b)

            # ---- merged attention + QKV(g+1) + projection stream ----
            flat = []
            for g in range(NQG):
                nb = 2 * (g + 1)
                for h in range(HPC):
                    for pos in range(nb):
                        flat.append((g, h, pos, pos == nb - 1, pos == 0))
            state = {}
            pending = []  # (delay_in_batches, closure)
            fillers = {}  # batch index -> list of closures

            # distribute QKV(g+1) units across attention batches of group g
            mstart = {}
            mi = 0
            for g in range(NQG):
                mstart[g] = mi
                mi += 2 * (g + 1) * HPC
            for g in range(NQG - 1):
                units = p1_units(g + 1)
                nbat = 2 * (g + 1) * HPC
                for j, u in enumerate(units):
                    m = mstart[g] + min(nbat - 1, (j * nbat) // len(units))
                    fillers.setdefault(m, []).append(u)
            # wo load once the startup HWDGE burst has drained; xt3 early in
            # group-1's window (slot frees after group-0's V units read xt0)
            fillers.setdefault(mstart[0], []).insert(
                0,
                lambda: nc.sync.dma_start(
                    wo_sb, wo_d.rearrange("(h p) e -> p h e", p=128)
                ),
            )
            fillers.setdefault(mstart[1], []).insert(
                0, lambda: xts.append(issue_xt(3))
            )

            def nbatches(g):
                return 2 * (g + 1)

            def c0_of(g, kb):
                i = kb - 4 * g
                return 128 * i if i > 0 else 0

            def emit_S(m):
                g, h, j, last, first = flat[m]
                ps = ps_s.tile([128, 2, 512], F32, tag="ps")
                for t in range(2):
                    kb = 2 * j + t
                    c0 = 0 if j == 2 * g else c0_of(g, kb)
                    nc.tensor.matmul(
                        ps[:, t, c0:],
                        k_ts[h][:, kb * 128 : (kb + 1) * 128],
                        q_ts[h][:, g * 512 + c0 : (g + 1) * 512],
                        start=True, stop=True,
                    )
                state[m] = ps

            def emit_rest(m):
                g, h, j, last, first = flat[m]
                ps = state.pop(m)
                if first:
                    state[("po", g, h)] = ps_o.tile(
                        [128, 512], F32, tag="po", name="po"
                    )
                    state[("acc", g, h)] = pacc.tile(
                        [128, 512], BF16, tag="acc", name="acc"
                    )
                po = state[("po", g, h)]
                acc = state[("acc", g, h)]
                at = pat.tile([128, 2, 512], BF16, tag="at")
                if j == 2 * g + 1:
                    for t in range(2):
                        c0 = c0_of(g, 2 * j + t)
                        nc.scalar.activation(
                            at[:, t, c0:], ps[:, t, c0:], EXP, scale=SCALE
                        )
                else:
                    nc.scalar.activation(at, ps, EXP, scale=SCALE)
                for t in range(2):
                    kb = 2 * j + t
                    i = kb - 4 * g
                    c0 = c0_of(g, kb)
                    if i >= 0:
                        nc.vector.tensor_mul(
                            at[:, t, c0 : c0 + 128], at[:, t, c0 : c0 + 128], tri
                        )
                    if first and t == 0:
                        nc.vector.tensor_copy(acc, at[:, 0, :])
                    else:
                        nc.vector.tensor_add(acc[:, c0:], acc[:, c0:], at[:, t, c0:])
                    nc.tensor.matmul(
                        po[:, c0:],
                        v_t[:, kb, h * 128 : (h + 1) * 128],
                        at[:, t, c0:],
                        start=(first and t == 0), stop=(last and t == 1),
                    )

            def emit_par(g, h, lo, hi):
                # Pool-side partition reduce only (runs on the otherwise
                # idle gpsimd queue, displacing nothing)
                def run():
                    acc = state[("acc", g, h)]
                    if ("z", g, h) not in state:
                        state[("z", g, h)] = pz.tile(
                            [128, 512], F32, tag="z", name="z"
                        )
                    nc.gpsimd.partition_all_reduce(
                        state[("z", g, h)][:, lo:hi], acc[:, lo:hi], 128,
                        bass_isa.ReduceOp.add,
                    )
                return run

            def emit_norm(g, h, lo, hi, pop):
                # DVE-side reciprocal + normalize for a column range
                def run():
                    po = state[("po", g, h)]
                    z = state[("z", g, h)]
                    if pop:
                        state.pop(("po", g, h))
                        state.pop(("acc", g, h))
                        state.pop(("z", g, h))
                    recip = pn.tile([128, 512], F32, tag="recip")
                    nc.vector.reciprocal(recip[:, lo:hi], z[:, lo:hi])
                    nc.vector.tensor_mul(
                        o_ts[h][:, g * 512 + lo : g * 512 + hi],
                        po[:, lo:hi], recip[:, lo:hi],
                    )
                return run

            def emit_finalize(g, h, lo=0, hi=512, pop=True):
                par = emit_par(g, h, lo, hi)
                norm = emit_norm(g, h, lo, hi, pop)
                def run():
                    par()
                    norm()
                return run

            ysb_live = {}

            def proj_half(g, b, eh):
                def run():
                    lb = g * 4 + b
                    lsl = slice(lb * 128, (lb + 1) * 128)
                    if (g, b) not in ysb_live:
                        ysb_live[(g, b)] = pysb.tile(
                            [128, 2, 384], BF16, tag="ysb", name="ysb"
                        )
                    ysb = ysb_live[(g, b)]
                    pyp = ps_g.tile([128, 512], F32, tag="gemm", name="pg")
                    for h2 in range(HPC):
                        nc.tensor.matmul(
                            pyp[:, :384],
                            o_ts[h2][:, lsl],
                            wo_sb[:, h2, eh * 384 : (eh + 1) * 384],
                            start=(h2 == 0), stop=(h2 == HPC - 1),
                        )
                    # alternate DVE/ACT so back-to-back projection copies
                    # run in parallel
                    if eh == 0:
                        nc.vector.tensor_copy(ysb[:, eh, :], pyp[:, :384])
                    else:
                        nc.scalar.activation(
                            ysb[:, eh, :], pyp[:, :384],
                            mybir.ActivationFunctionType.Copy,
                        )
                        nc.sync.dma_start(
                            y_d[lb * 128 : (lb + 1) * 128, :].rearrange(
                                "p (u e) -> p u e", u=2
                            ),
                            ysb,
                        )
                        del ysb_live[(g, b)]
                return run

            emit_S(0)
            for m in range(len(flat)):
                if m + 1 < len(flat):
                    emit_S(m + 1)
                nxt = []
                for d, fn in pending:
                    if d <= 0:
                        fn()
                    else:
                        nxt.append((d - 1, fn))
                pending = nxt
                for u in fillers.get(m, ()):
                    u()
                emit_rest(m)
                g, h, j, last, first = flat[m]
                lasthead = g == NQG - 1 and h == HPC - 1
                if lasthead and j == nbatches(g) - 2:
                    # columns [0:256) of acc are complete one batch early
                    # (the final diagonal batch only touches cols >= 256):
                    # run their partition-reduce concurrently on gpsimd
                    pending.append((1, emit_par(g, h, 0, 256)))
                if last:
                    if lasthead:
                        pending.append((1, emit_norm(g, h, 0, 256, False)))
                        pending.append((1, emit_finalize(g, h, 256, 512)))
                    else:
                        pending.append((1, emit_finalize(g, h)))
                    if h == HPC - 1 and g < NQG - 1:
                        # spread the projection half-units across the next
                        # group's batches: they are the PE filler that
                        # absorbs the per-batch ACT exp overhead deficit
                        nnext = 2 * (g + 2) * HPC
                        for i, (b, eh) in enumerate(
                            (b, eh) for b in range(4) for eh in range(2)
                        ):
                            pending.append(
                                (2 + (i * (nnext - 4)) // 8, proj_half(g, b, eh))
                            )
            # ---- tail: last group's projection, split by head so the
            # h0/h1 partial matmuls run during the final softmax chain
            # (borrowing the now-idle S-ring PSUM banks); emitted BEFORE
            # the flushed finalize so they sit earlier in the PE queue ----
            gl = NQG - 1
            pre = [(b, eh) for b in range(3) for eh in range(2)] + [(3, 0)]
            tgts = []
            for _ in range(2):
                tile_s = ps_s.tile([128, 2, 512], F32, tag="ps", name="pyA")
                tgts += [tile_s[:, 0, :384], tile_s[:, 1, :384]]
            for _ in range(2):
                tile_g = ps_g.tile([128, 512], F32, tag="gemm", name="pg")
                tgts.append(tile_g[:, :384])
            # 7th half in the po-ring slot freed by the previous finalize
            tile_o = ps_o.tile([128, 512], F32, tag="po", name="po")
            tgts.append(tile_o[:, :384])
            for (b, eh), tg in zip(pre, tgts):
                lsl = slice((4 * gl + b) * 128, (4 * gl + b + 1) * 128)
                for h2 in (0, 1):
                    nc.tensor.matmul(
                        tg, o_ts[h2][:, lsl],
                        wo_sb[:, h2, eh * 384 : (eh + 1) * 384],
                        start=(h2 == 0), stop=False,
                    )

            for d, fn in sorted(pending, key=lambda p: p[0]):
                fn()
            ysb4 = pp.tile([128, 4, 2, 384], BF16)

            def tail_copy(b, eh, tg):
                if eh == 0:
                    nc.vector.tensor_copy(ysb4[:, b, eh, :], tg)
                else:
                    nc.scalar.activation(
                        ysb4[:, b, eh, :], tg,
                        mybir.ActivationFunctionType.Copy,
                    )

            def tail_store(bpair, eng):
                r0 = (4 * gl + 2 * bpair) * 128
                eng.dma_start(
                    y_d[r0 : r0 + 256, :].rearrange(
                        "(b p) (u e) -> p b u e", p=128, u=2
                    ),
                    ysb4[:, 2 * bpair : 2 * bpair + 2],
                )

            for (b, eh), tg in zip(pre, tgts):
                lsl = slice((4 * gl + b) * 128, (4 * gl + b + 1) * 128)
                nc.tensor.matmul(
                    tg, o_ts[2][:, lsl], wo_sb[:, 2, eh * 384 : (eh + 1) * 384],
                    start=False, stop=True,
                )
                tail_copy(b, eh, tg)
                if b == 1 and eh == 1:
                    tail_store(0, nc.sync)
                if b == 2 and eh == 1:
                    r2 = (4 * gl + 2) * 128
                    nc.sync.dma_start(
                        y_d[r2 : r2 + 128, :].rearrange("p (u e) -> p u e", u=2),
                        ysb4[:, 2],
                    )
            # last half: full 3-matmul unit on a recycled gemm slot
            pyl = ps_g.tile([128, 512], F32, tag="gemm", name="pg")
            lsl = slice((4 * gl + 3) * 128, (4 * gl + 4) * 128)
            for h2 in range(HPC):
                nc.tensor.matmul(
                    pyl[:, :384], o_ts[h2][:, lsl],
                    wo_sb[:, h2, 384:768],
                    start=(h2 == 0), stop=(h2 == HPC - 1),
                )
            tail_copy(3, 1, pyl[:, :384])
            r3 = (4 * gl + 3) * 128
            nc.scalar.dma_start(
                y_d[r3 : r3 + 128, :].rearrange("p (u e) -> p u e", u=2),
                ysb4[:, 3],
            )

    nc.compile()
    return nc


_NC_CACHE = {}


def _get_nc(L_=L):
    if L_ not in _NC_CACHE:
        _NC_CACHE[L_] = build_nc(L_)
    return _NC_CACHE[L_]


def run_sharded(inputs, L_=L, trace=False):
    bf16 = ml_dtypes.bfloat16
    x = np.asarray(inputs["x_input"], dtype=np.float32).astype(bf16)
    xT = np.ascontiguousarray(x.transpose(0, 2, 1))  # [B, D, L]
    tri = np.triu(np.ones((128, 128), dtype=np.float32)).astype(bf16)
    wq = np.asarray(inputs["Wq"], np.float32).astype(bf16)
    wk = np.asarray(inputs["Wk"], np.float32).astype(bf16)
    wv = np.asarray(inputs["Wv"], np.float32).astype(bf16)
    wo = np.asarray(inputs["Wo"], np.float32).astype(bf16)
    bq = np.asarray(inputs["bq"], np.float32)
    bk = np.asarray(inputs["bk"], np.float32)
    in_maps = []
    for c in range(N_CORES):
        b, gslice = c // 2, slice((c % 2) * HG, (c % 2) * HG + HG)
        in_maps.append(
            {
                "xT": xT[b],
                "tri": tri,
                "wq": np.ascontiguousarray(wq[:, gslice]),
                "wk": np.ascontiguousarray(wk[:, gslice]),
                "wv": np.ascontiguousarray(wv[:, gslice]),
                "wo": np.ascontiguousarray(wo[gslice, :]),
                "bq": np.ascontiguousarray(bq[gslice]),
                "bk": np.ascontiguousarray(bk[gslice]),
            }
        )
    nc = _get_nc(L_)
    try:
        res = run_bass_kernel_spmd(nc, in_maps, list(range(N_CORES)), trace=trace)
    except Exception:
        res = run_bass_kernel_spmd(nc, in_maps, list(range(N_CORES)), trace=trace)
    return res


def kernel(**inputs) -> np.ndarray:
    res = run_sharded(inputs)
    bias = (
        np.asarray(inputs["bv"], np.float32) @ np.asarray(inputs["Wo"], np.float32)
        + np.asarray(inputs["bo"], np.float32)
    )
    out = np.empty((B, L, D), dtype=np.float32)
    for b in range(B):
        out[b] = (
            np.asarray(res.results[2 * b]["y"], dtype=np.float32)
            + np.asarray(res.results[2 * b + 1]["y"], dtype=np.float32)
            + bias
        )
    return out


# revision 54
# speedup vs baseline: 1.3024x; 1.0004x over previous
"""Causal self-attention for B=4, L=2048, D=768, H=6 on 8 TRN2 NeuronCores.

Sharding: 8 cores = 4 batches x 2 head-groups (3 heads / 384 hidden each).
All device math bf16 (fp32 PSUM accumulation), single merged pipeline:
the QKV projection of q-group g+1 runs as PE filler inside the attention
stream of q-group g, so the ACT exp latency/throughput never exposes PE
idle. x^T is pre-transposed on the host. Softmax denominators via DVE
bf16 accumulation (2x perf mode) + GPSIMD partition_all_reduce (no PE
pass, no PSUM bank); diagonal-block triangle masks on DVE; projection
staged PSUM->SBUF (DVE/ACT alternating, bf16) then DMA'd; the last
group's projection is split by head so its h0/h1 partials overlap the
final softmax-normalization chain. Dummy warm-up matmuls bridge the
startup DMA latency so the PE p-state ramp is warm when real work lands.
Host sums the two head-group partials per batch and adds bv@Wo + bo
(softmax rows sum to 1, so the bv term commutes out exactly).
"""

import math

import numpy as np
import ml_dtypes

import concourse.bacc as bacc
import concourse.bass_isa as bass_isa
import concourse.mybir as mybir
import concourse.tile as tile
from concourse.bass_utils import run_bass_kernel_spmd

F32 = mybir.dt.float32
BF16 = mybir.dt.bfloat16
EXP = mybir.ActivationFunctionType.Exp
IDENT = mybir.ActivationFunctionType.Identity

B = 4
L = 2048
D = 768
HEADS = 6
HD = 128
HPC = 3          # heads per core
HG = HPC * HD    # 384: per-core slice of the hidden dim
CB = D // 128    # 6 contraction chunks
SCALE = 1.0 / math.sqrt(HD)
N_CORES = 8


def build_nc(L_=L):
    NQG = L_ // 512   # 512-wide q groups

    nc = bacc.Bacc("TRN2", target_bir_lowering=False, debug=False)
    xT_d = nc.dram_tensor("xT", [D, L_], BF16, kind="ExternalInput").ap()
    wq_d = nc.dram_tensor("wq", [D, HG], BF16, kind="ExternalInput").ap()
    wk_d = nc.dram_tensor("wk", [D, HG], BF16, kind="ExternalInput").ap()
    wv_d = nc.dram_tensor("wv", [D, HG], BF16, kind="ExternalInput").ap()
    wo_d = nc.dram_tensor("wo", [HG, D], BF16, kind="ExternalInput").ap()
    bq_d = nc.dram_tensor("bq", [HG], F32, kind="ExternalInput").ap()
    bk_d = nc.dram_tensor("bk", [HG], F32, kind="ExternalInput").ap()
    tri_d = nc.dram_tensor("tri", [128, 128], BF16, kind="ExternalInput").ap()
    y_d = nc.dram_tensor("y", [L_, D], BF16, kind="ExternalOutput").ap()

    xT_r = xT_d.rearrange("(c p) l -> p c l", p=128)

    with tile.TileContext(nc) as tc:
        with (
            tc.tile_pool(name="persist", bufs=1) as pp,
            tc.tile_pool(name="qkv_sb", bufs=1) as pqkv,
            tc.tile_pool(name="xT", bufs=3) as pxt,
            tc.tile_pool(name="at_pool", bufs=8) as pat,
            tc.tile_pool(name="acc_pool", bufs=3) as pacc,
            tc.tile_pool(name="z_pool", bufs=2) as pz,
            tc.tile_pool(name="nrm_sb", bufs=2) as pn,
            tc.tile_pool(name="ysb_pool", bufs=3) as pysb,
            tc.tile_pool(name="ps_g", bufs=2, space="PSUM") as ps_g,
            tc.tile_pool(name="ps_s", bufs=2, space="PSUM") as ps_s,
            tc.tile_pool(name="ps_o", bufs=2, space="PSUM") as ps_o,
        ):
            # warmup scrap first: Pool boots earliest and must memset this
            # before its SWDGE descriptor-generation work queues up
            scrap = pp.tile([128, 128], BF16)
            nc.gpsimd.memset(scrap, 0.0)
            # small constants on the SWDGE (gpsimd) queue
            bq_sb = pp.tile([128, HPC], F32)
            bk_sb = pp.tile([128, HPC], F32)
            nc.gpsimd.dma_start(bq_sb, bq_d.rearrange("(h p) -> p h", p=128))
            nc.gpsimd.dma_start(bk_sb, bk_d.rearrange("(h p) -> p h", p=128))
            # dummy exp: pulls the ACT Exp-table load into the startup shadow
            warm = pp.tile([1, 1], F32)
            nc.scalar.activation(warm, bq_sb[:1, :1], EXP)

            # per-head tensors as separate tiles: keeps the scheduler's
            # dependency tracking precise across heads
            q_ts = [pqkv.tile([128, L_], BF16, name=f"q_t{h}") for h in range(HPC)]
            k_ts = [pqkv.tile([128, L_], BF16, name=f"k_t{h}") for h in range(HPC)]
            v_t = pqkv.tile([128, L_ // 128, HG], BF16)
            o_ts = [pqkv.tile([128, L_], BF16, name=f"o_t{h}") for h in range(HPC)]

            wq_sb = pp.tile([128, CB, HG], BF16)
            wk_sb = pp.tile([128, CB, HG], BF16)
            wv_sb = pp.tile([128, CB, HG], BF16)
            wo_sb = pp.tile([128, HPC, D], BF16)
            tri = pp.tile([128, 128], BF16)

            # ---- startup DMAs: wq/xt0 in halves, rest whole; wo deferred ----
            xts = []
            xt0 = pxt.tile([128, CB, 512], BF16, tag="xt", name="xt")
            wq_r = wq_d.rearrange("(c p) d -> p c d", p=128)
            for half in range(2):
                cs = slice(3 * half, 3 * half + 3)
                nc.scalar.dma_start(wq_sb[:, cs, :], wq_r[:, cs, :])
                nc.sync.dma_start(xt0[:, cs, :], xT_r[:, cs, 0:512])
            xts.append(xt0)

            def issue_xt(g):
                xt = pxt.tile([128, CB, 512], BF16, tag="xt", name="xt")
                nc.sync.dma_start(xt, xT_r[:, :, g * 512 : (g + 1) * 512])
                return xt

            nc.scalar.dma_start(wk_sb, wk_d.rearrange("(c p) d -> p c d", p=128))
            xts.append(issue_xt(1))
            nc.scalar.dma_start(wv_sb, wv_d.rearrange("(c p) d -> p c d", p=128))
            xts.append(issue_xt(2))
            nc.gpsimd.dma_start(tri, tri_d)

            # ---- phase-1 unit emitters ----
            def emit_qk_unit(g, h, which):
                w_sb, t_sb, b_sb = (
                    (wq_sb, q_ts[h], bq_sb) if which == "q" else (wk_sb, k_ts[h], bk_sb)
                )
                hsl = slice(h * 128, (h + 1) * 128)
                pq = ps_g.tile([128, 512], F32, tag="gemm", name="pg")
                for c in range(CB):
                    nc.tensor.matmul(
                        pq, w_sb[:, c, hsl], xts[g][:, c, :],
                        start=(c == 0), stop=(c == CB - 1),
                    )
                nc.scalar.activation(
                    t_sb[:, g * 512 : (g + 1) * 512], pq, IDENT,
                    bias=b_sb[:, h : h + 1],
                )

            def emit_v_unit(g, b):
                lb = g * 4 + b
                pv = ps_g.tile([128, 512], F32, tag="gemm", name="pg")
                for c in range(CB):
                    nc.tensor.matmul(
                        pv[:, :HG], xts[g][:, c, b * 128 : (b + 1) * 128],
                        wv_sb[:, c, :],
                        start=(c == 0), stop=(c == CB - 1),
                    )
                nc.vector.tensor_copy(v_t[:, lb, :], pv[:, :HG])

            def p1_units(g):
                units = []
                for h in range(HPC):
                    units.append(lambda g=g, h=h: emit_qk_unit(g, h, "q"))
                    units.append(lambda g=g, h=h: emit_qk_unit(g, h, "k"))
                for b in range(4):
                    units.append(lambda g=g, b=b: emit_v_unit(g, b))
                return units

            # PE warmup: dummy matmuls on a memset tile keep the tensor
            # engine's p-state ramp alive while the startup DMAs trickle in
            # (any PE idle gap resets the ramp to the slow p-state)
            # scrap PSUM target: first slot of the (startup-idle) S ring
            ps_w = ps_s.tile([128, 2, 512], F32, tag="ps", name="warm")

            def pe_fill(n):
                # n counted in 512-col equivalents; emit 128-col dummies for
                # finer-grained bridging of the startup DMA pacing
                for _ in range(4 * n):
                    nc.tensor.matmul(
                        ps_w[:, 0, :128], scrap, scrap,
                        start=True, stop=True, skip_group_check=True,
                    )

            pe_fill(5)

            # ---- group 0 QKV: chunk-major q (2 heads) so matmuls start as
            # soon as the first wq/xT chunks land ----
            pqs = [ps_g.tile([128, 512], F32, tag="gemm", name="pg")
                   for _ in range(2)]
            for c in range(CB):
                if c == 3:
                    # bridge the gap until the second wq/xT halves land
                    pe_fill(2)
                for h in range(2):
                    nc.tensor.matmul(
                        pqs[h], wq_sb[:, c, h * 128 : (h + 1) * 128],
                        xts[0][:, c, :],
                        start=(c == 0), stop=(c == CB - 1),
                        skip_group_check=True,
                    )
                pe_fill(1)
            for h in range(2):
                nc.scalar.activation(
                    q_ts[h][:, 0:512], pqs[h], IDENT, bias=bq_sb[:, h : h + 1]
                )
            del pqs
            pe_fill(2)
            emit_qk_unit(0, 2, "q")
            for h in range(HPC):
                emit_qk_unit(0, h, "k")
            for b in range(4):
                emit_v_unit(0, b)

            # ---- merged attention + QKV(g+1) + projection stream ----
            flat = []
            for g in range(NQG):
                nb = 2 * (g + 1)
                for h in range(HPC):
                    for pos in range(nb):
                        flat.append((g, h, pos, pos == nb - 1, pos == 0))
            state = {}
            pending = []  # (delay_in_batches, closure)
            fillers = {}  # batch index -> list of closures

            # distribute QKV(g+1) units across attention batches of group g
            mstart = {}
            mi = 0
            for g in range(NQG):
                mstart[g] = mi
                mi += 2 * (g + 1) * HPC
            for g in range(NQG - 1):
                units = p1_units(g + 1)
                nbat = 2 * (g + 1) * HPC
                for j, u in enumerate(units):
                    m = mstart[g] + min(nbat - 1, (j * nbat) // len(units))
                    fillers.setdefault(m, []).append(u)
            # wo load once the startup HWDGE burst has drained; xt3 early in
            # group-1's window (slot frees after group-0's V units read xt0)
            fillers.setdefault(mstart[0], []).insert(
                0,
                lambda: nc.sync.dma_start(
                    wo_sb, wo_d.rearrange("(h p) e -> p h e", p=128)
                ),
            )
            fillers.setdefault(mstart[1], []).insert(
                0, lambda: xts.append(issue_xt(3))
            )

            def nbatches(g):
                return 2 * (g + 1)

            def c0_of(g, kb):
                i = kb - 4 * g
                return 128 * i if i > 0 else 0

            def emit_S(m):
                g, h, j, last, first = flat[m]
                ps = ps_s.tile([128, 2, 512], F32, tag="ps")
                for t in range(2):
                    kb = 2 * j + t
                    c0 = 0 if j == 2 * g else c0_of(g, kb)
                    nc.tensor.matmul(
                        ps[:, t, c0:],
                        k_ts[h][:, kb * 128 : (kb + 1) * 128],
                        q_ts[h][:, g * 512 + c0 : (g + 1) * 512],
                        start=True, stop=True,
                    )
                state[m] = ps

            def emit_rest(m):
                g, h, j, last, first = flat[m]
                ps = state.pop(m)
                if first:
                    state[("po", g, h)] = ps_o.tile(
                        [128, 512], F32, tag="po", name="po"
                    )
                    state[("acc", g, h)] = pacc.tile(
                        [128, 512], BF16, tag="acc", name="acc"
                    )
                po = state[("po", g, h)]
                acc = state[("acc", g, h)]
                at = pat.tile([128, 2, 512], BF16, tag="at")
                if j == 2 * g + 1:
                    for t in range(2):
                        c0 = c0_of(g, 2 * j + t)
                        nc.scalar.activation(
                            at[:, t, c0:], ps[:, t, c0:], EXP, scale=SCALE
                        )
                else:
                    nc.scalar.activation(at, ps, EXP, scale=SCALE)
                for t in range(2):
                    kb = 2 * j + t
                    i = kb - 4 * g
                    c0 = c0_of(g, kb)
                    if i >= 0:
                        nc.vector.tensor_mul(
                            at[:, t, c0 : c0 + 128], at[:, t, c0 : c0 + 128], tri
                        )
                    if first and t == 0:
                        nc.vector.tensor_copy(acc, at[:, 0, :])
                    else:
                        nc.vector.tensor_add(acc[:, c0:], acc[:, c0:], at[:, t, c0:])
                    nc.tensor.matmul(
                        po[:, c0:],
                        v_t[:, kb, h * 128 : (h + 1) * 128],
                        at[:, t, c0:],
                        start=(first and t == 0), stop=(last and t == 1),
                    )

            def emit_par(g, h, lo, hi):
                # Pool-side partition reduce only (runs on the otherwise
                # idle gpsimd queue, displacing nothing)
                def run():
                    acc = state[("acc", g, h)]
                    if ("z", g, h) not in state:
                        state[("z", g, h)] = pz.tile(
                            [128, 512], F32, tag="z", name="z"
                        )
                    nc.gpsimd.partition_all_reduce(
                        state[("z", g, h)][:, lo:hi], acc[:, lo:hi], 128,
                        bass_isa.ReduceOp.add,
                    )
                return run

            def emit_norm(g, h, lo, hi, pop):
                # DVE-side reciprocal + normalize for a column range
                def run():
                    po = state[("po", g, h)]
                    z = state[("z", g, h)]
                    if pop:
                        state.pop(("po", g, h))
                        state.pop(("acc", g, h))
                        state.pop(("z", g, h))
                    recip = pn.tile([128, 512], F32, tag="recip")
                    nc.vector.reciprocal(recip[:, lo:hi], z[:, lo:hi])
                    nc.vector.tensor_mul(
                        o_ts[h][:, g * 512 + lo : g * 512 + hi],
                        po[:, lo:hi], recip[:, lo:hi],
                    )
                return run

            def emit_finalize(g, h, lo=0, hi=512, pop=True):
                par = emit_par(g, h, lo, hi)
                norm = emit_norm(g, h, lo, hi, pop)
                def run():
                    par()
                    norm()
                return run

            ysb_live = {}

            def proj_half(g, b, eh):
                def run():
                    lb = g * 4 + b
                    lsl = slice(lb * 128, (lb + 1) * 128)
                    if (g, b) not in ysb_live:
                        ysb_live[(g, b)] = pysb.tile(
                            [128, 2, 384], BF16, tag="ysb", name="ysb"
                        )
                    ysb = ysb_live[(g, b)]
                    pyp = ps_g.tile([128, 512], F32, tag="gemm", name="pg")
                    for h2 in range(HPC):
                        nc.tensor.matmul(
                            pyp[:, :384],
                            o_ts[h2][:, lsl],
                            wo_sb[:, h2, eh * 384 : (eh + 1) * 384],
                            start=(h2 == 0), stop=(h2 == HPC - 1),
                        )
                    # alternate DVE/ACT so back-to-back projection copies
                    # run in parallel
                    if eh == 0:
                        nc.vector.tensor_copy(ysb[:, eh, :], pyp[:, :384])
                    else:
                        nc.scalar.activation(
                            ysb[:, eh, :], pyp[:, :384],
                            mybir.ActivationFunctionType.Copy,
                        )
                        nc.sync.dma_start(
                            y_d[lb * 128 : (lb + 1) * 128, :].rearrange(
                                "p (u e) -> p u e", u=2
                            ),
                            ysb,
                        )
                        del ysb_live[(g, b)]
                return run

            emit_S(0)
            for m in range(len(flat)):
                if m + 1 < len(flat):
                    emit_S(m + 1)
                nxt = []
                for d, fn in pending:
                    if d <= 0:
                        fn()
                    else:
                        nxt.append((d - 1, fn))
                pending = nxt
                for u in fillers.get(m, ()):
                    u()
                emit_rest(m)
                g, h, j, last, first = flat[m]
                lasthead = g == NQG - 1 and h == HPC - 1
                if lasthead and j == nbatches(g) - 2:
                    # columns [0:256) of acc are complete one batch early
                    # (the final diagonal batch only touches cols >= 256):
                    # run their partition-reduce concurrently on gpsimd
                    pending.append((1, emit_par(g, h, 0, 256)))
                if last:
                    if lasthead:
                        pending.append((1, emit_norm(g, h, 0, 256, False)))
                        pending.append((1, emit_finalize(g, h, 256, 512)))
                    else:
                        pending.append((1, emit_finalize(g, h)))
                    if h == HPC - 1 and g < NQG - 1:
                        # spread the projection half-units across the next
                        # group's batches: they are the PE filler that
                        # absorbs the per-batch ACT exp overhead deficit
                        nnext = 2 * (g + 2) * HPC
                        for i, (b, eh) in enumerate(
                            (b, eh) for b in range(4) for eh in range(2)
                        ):
                            pending.append(
                                (2 + (i * (nnext - 4)) // 8, proj_half(g, b, eh))
                            )
            # ---- tail: last group's projection, split by head so the
            # h0/h1 partial matmuls run during the final softmax chain
            # (borrowing the now-idle S-ring PSUM banks); emitted BEFORE
            # the flushed finalize so they sit earlier in the PE queue ----
            gl = NQG - 1
            pre = [(b, eh) for b in range(3) for eh in range(2)] + [(3, 0)]
            tgts = []
            for _ in range(2):
                tile_s = ps_s.tile([128, 2, 512], F32, tag="ps", name="pyA")
                tgts += [tile_s[:, 0, :384], tile_s[:, 1, :384]]
            for _ in range(2):
                tile_g = ps_g.tile([128, 512], F32, tag="gemm", name="pg")
                tgts.append(tile_g[:, :384])
            # 7th half in the po-ring slot freed by the previous finalize
            tile_o = ps_o.tile([128, 512], F32, tag="po", name="po")
            tgts.append(tile_o[:, :384])
            for (b, eh), tg in zip(pre, tgts):
                lsl = slice((4 * gl + b) * 128, (4 * gl + b + 1) * 128)
                for h2 in (0, 1):
                    nc.tensor.matmul(
                        tg, o_ts[h2][:, lsl],
                        wo_sb[:, h2, eh * 384 : (eh + 1) * 384],
                        start=(h2 == 0), stop=False,
                    )

            for d, fn in sorted(pending, key=lambda p: p[0]):
                fn()
            ysb4 = pp.tile([128, 4, 2, 384], BF16)

            def tail_copy(b, eh, tg):
                if eh == 0:
                    nc.vector.tensor_copy(ysb4[:, b, eh, :], tg)
                else:
                    nc.scalar.activation(
                        ysb4[:, b, eh, :], tg,
                        mybir.ActivationFunctionType.Copy,
                    )

            def tail_store(bpair, eng):
                r0 = (4 * gl + 2 * bpair) * 128
                eng.dma_start(
                    y_d[r0 : r0 + 256, :].rearrange(
                        "(b p) (u e) -> p b u e", p=128, u=2
                    ),
                    ysb4[:, 2 * bpair : 2 * bpair + 2],
                )

            for (b, eh), tg in zip(pre, tgts):
                lsl = slice((4 * gl + b) * 128, (4 * gl + b + 1) * 128)
                nc.tensor.matmul(
                    tg, o_ts[2][:, lsl], wo_sb[:, 2, eh * 384 : (eh + 1) * 384],
                    start=False, stop=True,
                )
                tail_copy(b, eh, tg)
                if b == 1 and eh == 1:
                    tail_store(0, nc.sync)
                if b == 2 and eh == 1:
                    r2 = (4 * gl + 2) * 128
                    nc.sync.dma_start(
                        y_d[r2 : r2 + 128, :].rearrange("p (u e) -> p u e", u=2),
                        ysb4[:, 2],
                    )
            # last half: full 3-matmul unit on a recycled gemm slot
            pyl = ps_g.tile([128, 512], F32, tag="gemm", name="pg")
            lsl = slice((4 * gl + 3) * 128, (4 * gl + 4) * 128)
            for h2 in range(HPC):
                nc.tensor.matmul(
                    pyl[:, :384], o_ts[h2][:, lsl],
                    wo_sb[:, h2, 384:768],
                    start=(h2 == 0), stop=(h2 == HPC - 1),
                )
            tail_copy(3, 1, pyl[:, :384])
            r3 = (4 * gl + 3) * 128
            nc.scalar.dma_start(
                y_d[r3 : r3 + 128, :].rearrange("p (u e) -> p u e", u=2),
                ysb4[:, 3],
            )

    nc.compile()
    return nc


_NC_CACHE = {}


def _get_nc(L_=L):
    if L_ not in _NC_CACHE:
        _NC_CACHE[L_] = build_nc(L_)
    return _NC_CACHE[L_]


def run_sharded(inputs, L_=L, trace=False):
    bf16 = ml_dtypes.bfloat16
    x = np.asarray(inputs["x_input"], dtype=np.float32).astype(bf16)
    xT = np.ascontiguousarray(x.transpose(0, 2, 1))  # [B, D, L]
    tri = np.triu(np.ones((128, 128), dtype=np.float32)).astype(bf16)
    wq = np.asarray(inputs["Wq"], np.float32).astype(bf16)
    wk = np.asarray(inputs["Wk"], np.float32).astype(bf16)
    wv = np.asarray(inputs["Wv"], np.float32).astype(bf16)
    wo = np.asarray(inputs["Wo"], np.float32).astype(bf16)
    bq = np.asarray(inputs["bq"], np.float32)
    bk = np.asarray(inputs["bk"], np.float32)
    in_maps = []
    for c in range(N_CORES):
        b, gslice = c // 2, slice((c % 2) * HG, (c % 2) * HG + HG)
        in_maps.append(
            {
                "xT": xT[b],
                "tri": tri,
                "wq": np.ascontiguousarray(wq[:, gslice]),
                "wk": np.ascontiguousarray(wk[:, gslice]),
                "wv": np.ascontiguousarray(wv[:, gslice]),
                "wo": np.ascontiguousarray(wo[gslice, :]),
                "bq": np.ascontiguousarray(bq[gslice]),
                "bk": np.ascontiguousarray(bk[gslice]),
            }
        )
    nc = _get_nc(L_)
    try:
        res = run_bass_kernel_spmd(nc, in_maps, list(range(N_CORES)), trace=trace)
    except Exception:
        res = run_bass_kernel_spmd(nc, in_maps, list(range(N_CORES)), trace=trace)
    return res


def kernel(**inputs) -> np.ndarray:
    res = run_sharded(inputs)
    bias = (
        np.asarray(inputs["bv"], np.float32) @ np.asarray(inputs["Wo"], np.float32)
        + np.asarray(inputs["bo"], np.float32)
    )
    out = np.empty((B, L, D), dtype=np.float32)
    for b in range(B):
        out[b] = (
            np.asarray(res.results[2 * b]["y"], dtype=np.float32)
            + np.asarray(res.results[2 * b + 1]["y"], dtype=np.float32)
            + bias
        )
    return out
